# revision 7
# baseline (speedup 1.0000x reference)
"""Trainium2 Bass kernel for nn_Block1 (axial cross-attention block).

Sharding: pure data-parallel across 8 NeuronCores.
  - attn_h: each core takes a 16-wide w-range (all b, full h).
  - attn_w: each core takes a 16-tall h-range (all b, full w).
  - conv3x3 + MLP: each core takes a 16-tall h-range of output rows.
Three NEFF launches with host-side resharding between them.

Device kernels keep softmax un-normalized: exp(scores^T) [j,i] tiles are
DMA'd out directly plus per-row reciprocals; the host does amap = exp.T * rcp.
LayerNorm stats (rstd/-mu*rstd per pixel) are computed host-side (a tiny
reduction); the apply (x*alpha + beta) runs on-device (DVE mult + DMA
accumulate-add). LN gamma/beta are folded into consuming conv weights.
"""

import functools
import numpy as np

import concourse.bass as bass
import concourse.tile as tile
from concourse import mybir
from concourse.bass_utils import run_bass_kernel_spmd

# ---- walrus sync-wait workaround ----
import bass_rust
from concourse.vector_clock import ScopedClock

_MAXW = 1
_patch_state = {"done": False, "ctr": 0}


def _mk_nop(engine, wait):
    nop = bass_rust.InstNoOp(name=f"I-waitsplit-{_patch_state['ctr']}", ins=[], outs=[])
    _patch_state["ctr"] += 1
    nop.engine = engine
    nop.sync_info = bass_rust.SyncInfo(on_wait=[wait], on_update=[])
    return nop


def _split_list(insts):
    out = []
    changed = False
    for inst in insts:
        si = getattr(inst, "sync_info", None)
        waits = list(si.on_wait) if (si is not None and si.on_wait) else []
        if len(waits) > _MAXW:
            for w in waits[:-_MAXW]:
                out.append(_mk_nop(inst.engine, w))
            si.on_wait = waits[-_MAXW:]
            changed = True
        out.append(inst)
    if changed:
        insts[:] = out


def _patch_tile():
    if _patch_state["done"]:
        return
    _patch_state["done"] = True

    orig_lower = tile.TileContext._lower_ordered_insts

    def _lower_ordered_insts(self, ordered):
        for insts in ordered.values():
            _split_list(insts)
        return orig_lower(self, ordered)

    tile.TileContext._lower_ordered_insts = _lower_ordered_insts

    def _drain_and_barrier(self, tick_clock, wait_clock):
        drain_inst = self.nc.sync.drain()
        wait_clock.add_sem_waits(
            drain_inst.ins, ScopedClock({None: tick_clock.global_clock})
        )
        si = drain_inst.ins.sync_info
        waits = list(si.on_wait or [])
        if len(waits) > 1:
            si.on_wait = waits[:1]
            for w in waits[1:]:
                d2 = self.nc.sync.drain()
                si2 = d2.ins.sync_info
                if si2 is None:
                    d2.ins.sync_info = bass_rust.SyncInfo(on_wait=[w], on_update=[])
                else:
                    si2.on_wait = [w]

        self.nc.all_engine_barrier()
        assert self.sems is not None
        popped = self.nc._tile_sem_poison_stack.pop()
        assert popped is self._sem_poison
        self.nc.clear_and_free_semaphores(list(self.sems.allocated().values()))
        self.nc.all_engine_barrier()

    tile.TileContext._drain_and_barrier = _drain_and_barrier


_patch_tile()


def _install_neff_cache():
    """Disk-cache walrus NEFF compiles keyed on the BIR json hash."""
    import hashlib
    import os
    import shutil
    from concourse import bass2jax

    if getattr(bass2jax, "_neff_cache_installed", False):
        return
    bass2jax._neff_cache_installed = True
    orig = bass2jax.compile_bir_kernel
    cache_dir = os.path.expanduser("~/.bass_neff_cache")
    os.makedirs(cache_dir, exist_ok=True)

    def cached(bir_json, tmpdir, neff_name="file.neff"):
        key = hashlib.sha256(bir_json).hexdigest()
        p = os.path.join(cache_dir, key + ".neff")
        tgt = os.path.join(tmpdir, neff_name)
        if os.path.exists(p):
            shutil.copy(p, tgt)
            return tgt
        f = orig(bir_json, tmpdir, neff_name=neff_name)
        try:
            shutil.copy(f, p)
        except OSError:
            pass
        return f

    bass2jax.compile_bir_kernel = cached


_install_neff_cache()

# ---- problem constants ----
B, C, H, W = 4, 64, 128, 128
HEADS, DH = 8, 8
HID = 256
NC_ = 8
SL = 16
EPS = 1e-5
SCALE = C ** -0.5
F32 = mybir.dt.float32
AL = mybir.AluOpType
AF = mybir.ActivationFunctionType

PIX1 = H * SL              # 2048
ROWS3 = 22
W3PAD = W + 6              # 134
PIX3 = ROWS3 * W3PAD       # 2948
OUTR = 16
CATR = 24
CATW = 130


def _bc(t, parts, n, offset=0):
    return bass.AP(t, offset, [[0, parts], [1, n]])


def _fap(tile_t, nparts, offset, dims):
    """AP into a pool tile: partitions [0, nparts), free dims given explicitly."""
    a = tile_t[:nparts, :]
    return bass.AP(a.tensor, a.offset + offset, [a.ap[0]] + dims)


def _pslice(tile_t, p0, np_, offset, dims):
    """AP into tile partitions [p0, p0+np_) with explicit free dims."""
    a = tile_t[p0:p0 + np_, :]
    return bass.AP(a.tensor, a.offset + offset, [a.ap[0]] + dims)


# ================= kernel 1: layernorms + both attention branches ==========

@functools.lru_cache(maxsize=1)
def _build_k1():
    nc = bass.Bass()

    ins = {}
    for br in ("h", "w"):
        ins[f"xq_{br}"] = nc.dram_tensor(f"xq_{br}", [B, C, PIX1], F32, kind="ExternalInput")
        ins[f"xkv_{br}"] = nc.dram_tensor(f"xkv_{br}", [B, C, PIX1], F32, kind="ExternalInput")
        for nm in ("aq", "bq", "akv", "bkv"):
            ins[f"{nm}_{br}"] = nc.dram_tensor(f"{nm}_{br}", [B, 1, PIX1], F32, kind="ExternalInput")
        for nm in ("qT", "kT", "vT", "oT"):
            ins[f"{nm}_{br}"] = nc.dram_tensor(f"{nm}_{br}", [C + 1, C], F32, kind="ExternalInput")
    ident = nc.dram_tensor("ident", [128, 128], F32, kind="ExternalInput")

    outs = {}
    for br in ("h", "w"):
        outs[f"eamap_{br}"] = nc.dram_tensor(f"eamap_{br}", [B, SL, HEADS, 128, 128], F32, kind="ExternalOutput")
        outs[f"rcp_{br}"] = nc.dram_tensor(f"rcp_{br}", [B, SL, 128, HEADS], F32, kind="ExternalOutput")
        outs[f"outres_{br}"] = nc.dram_tensor(f"outres_{br}", [B, C, PIX1], F32, kind="ExternalOutput")

    with tile.TileContext(nc) as tc:
        import contextlib
        ctx = contextlib.ExitStack()
        with ctx:
            const = ctx.enter_context(tc.tile_pool(name="const", bufs=1))
            lnp = ctx.enter_context(tc.tile_pool(name="lnp", bufs=2))
            sbp = ctx.enter_context(tc.tile_pool(name="sbp", bufs=3))
            expp = ctx.enter_context(tc.tile_pool(name="expp", bufs=4))
            ps_qk = ctx.enter_context(tc.tile_pool(name="ps_qk", bufs=2, space="PSUM"))
            ps_vo = ctx.enter_context(tc.tile_pool(name="ps_vo", bufs=1, space="PSUM"))
            ps_sT = ctx.enter_context(tc.tile_pool(name="ps_sT", bufs=2, space="PSUM"))
            ps_sm = ctx.enter_context(tc.tile_pool(name="ps_sm", bufs=1, space="PSUM"))
            ps_oa = ctx.enter_context(tc.tile_pool(name="ps_oa", bufs=1, space="PSUM"))

            ident_sb = const.tile([128, 128], F32)
            nc.sync.dma_start(out=ident_sb, in_=ident[:, :])

            wsb = {}
            for br in ("h", "w"):
                for nm in ("qT", "kT", "vT", "oT"):
                    t = const.tile([C + 1, C], F32, tag=f"w_{nm}_{br}")
                    nc.sync.dma_start(out=t, in_=ins[f"{nm}_{br}"][:, :])
                    wsb[f"{nm}_{br}"] = t

            for b in range(B):
                for br in ("h", "w"):
                    if br == "h":
                        pstep, sstep = SL, 1      # col = 16*pix + s
                    else:
                        pstep, sstep = 1, 128     # col = 128*s + pix

                    # ---- LN applies ----
                    xq_raw = lnp.tile([C, PIX1], F32, tag="xq_raw")
                    nc.sync.dma_start(out=xq_raw, in_=ins[f"xq_{br}"][b, :, :])
                    a_bc = lnp.tile([C, PIX1], F32, tag="a_bc")
                    nc.sync.dma_start(out=a_bc, in_=_bc(ins[f"aq_{br}"], C, PIX1, b * PIX1))
                    xnq = lnp.tile([C + 1, PIX1], F32, tag="xnq")
                    nc.vector.tensor_mul(out=xnq[:C, :], in0=xq_raw, in1=a_bc)
                    nc.gpsimd.dma_start(out=xnq[:C, :], in_=_bc(ins[f"bq_{br}"], C, PIX1, b * PIX1), accum_op=AL.add)
                    nc.vector.memset(xnq[C:C + 1, :], 1.0)

                    xkv_raw = lnp.tile([C, PIX1], F32, tag="xkv_raw")
                    nc.sync.dma_start(out=xkv_raw, in_=ins[f"xkv_{br}"][b, :, :])
                    akv_bc = lnp.tile([C, PIX1], F32, tag="akv_bc")
                    nc.sync.dma_start(out=akv_bc, in_=_bc(ins[f"akv_{br}"], C, PIX1, b * PIX1))
                    xnkv = lnp.tile([C + 1, PIX1], F32, tag="xnkv")
                    nc.vector.tensor_mul(out=xnkv[:C, :], in0=xkv_raw, in1=akv_bc)
                    nc.gpsimd.dma_start(out=xnkv[:C, :], in_=_bc(ins[f"bkv_{br}"], C, PIX1, b * PIX1), accum_op=AL.add)
                    nc.vector.memset(xnkv[C:C + 1, :], 1.0)

                    qT_w, kT_w, vT_w, oT_w = (wsb[f"{nm}_{br}"] for nm in ("qT", "kT", "vT", "oT"))

                    for g in range(SL // 4):
                        s0 = 4 * g
                        rhs_q = _fap(xnq, C + 1, s0 * sstep, [[sstep, 4], [pstep, 128]])
                        rhs_kv = _fap(xnkv, C + 1, s0 * sstep, [[sstep, 4], [pstep, 128]])

                        qk_sb = {}
                        for nm, rhs, wT in (("q", rhs_q, qT_w), ("k", rhs_kv, kT_w)):
                            for half in range(2):
                                ps = ps_qk.tile([128, 512], F32, tag="qkspread")
                                for m in range(4):
                                    n_head = half * 4 + m
                                    nc.tensor.matmul(
                                        ps[32 * m:32 * m + 8, :],
                                        wT[:, 8 * n_head:8 * n_head + 8],
                                        rhs, start=True, stop=True,
                                        tile_position=(0, 32 * m),
                                    )
                                sb = sbp.tile([128, 512], F32, tag=f"{nm}sb{half}")
                                if nm == "q":
                                    nc.scalar.copy(out=sb, in_=ps)
                                else:
                                    nc.vector.tensor_copy(out=sb, in_=ps)
                                qk_sb[(nm, half)] = sb

                        v_ps = ps_vo.tile([C, 512], F32, tag="vps")
                        nc.tensor.matmul(v_ps, vT_w, rhs_kv, start=True, stop=True)
                        v_sb = sbp.tile([C, 512], F32, tag="vsb")
                        nc.vector.tensor_copy(out=v_sb, in_=v_ps)

                        o_ext = sbp.tile([C + 1, 512], F32, tag="oext")
                        nc.vector.memset(o_ext[C:C + 1, :], 1.0)

                        for ds in range(4):
                            s = s0 + ds
                            vT_ps = ps_sm.tile([128, C], F32, tag="small")
                            nc.tensor.transpose(vT_ps, _fap(v_sb, C, 128 * ds, [[1, 128]]), ident_sb[:C, :C])
                            vT_ext = sbp.tile([128, 9 * HEADS], F32, tag="vText")
                            nc.vector.memset(_fap(vT_ext, 128, 8, [[9, 8]]), 1.0)
                            nc.vector.tensor_copy(out=_fap(vT_ext, 128, 0, [[9, 8], [1, 8]]), in_=vT_ps)

                            o_all = ps_oa.tile([128, 9 * HEADS], F32, tag="oall")
                            for n_head in range(HEADS):
                                half, m = divmod(n_head, 4)
                                lhs = _pslice(qk_sb[("k", half)], 32 * m, 8, 128 * ds, [[1, 128]])
                                rhs = _pslice(qk_sb[("q", half)], 32 * m, 8, 128 * ds, [[1, 128]])
                                sT_ps = ps_sT.tile([128, 128], F32, tag="sT")
                                nc.tensor.matmul(sT_ps, lhs, rhs, start=True, stop=True,
                                                 tile_position=(32 * m, 0))
                                expT = expp.tile([128, 128], F32, tag="expT")
                                nc.scalar.activation(out=expT, in_=sT_ps, func=AF.Exp, scale=SCALE)
                                nc.sync.dma_start(out=outs[f"eamap_{br}"][b, s, n_head, :, :], in_=expT)
                                nc.tensor.matmul(o_all[:, 9 * n_head:9 * n_head + 9], expT,
                                                 vT_ext[:, 9 * n_head:9 * n_head + 9],
                                                 start=True, stop=True)

                            rcp_sb = sbp.tile([128, HEADS], F32, tag="rcp")
                            nc.vector.reciprocal(out=rcp_sb, in_=_fap(o_all, 128, 8, [[9, 8]]))
                            nc.sync.dma_start(out=outs[f"rcp_{br}"][b, s, :, :], in_=rcp_sb)

                            oT_sb = sbp.tile([128, C], F32, tag="oTsb")
                            nc.vector.tensor_mul(
                                out=_fap(oT_sb, 128, 0, [[8, 8], [1, 8]]),
                                in0=_fap(o_all, 128, 0, [[9, 8], [1, 8]]),
                                in1=_fap(rcp_sb, 128, 0, [[1, 8], [0, 8]]),
                            )

                            o_ps2 = ps_sm.tile([C, 128], F32, tag="small")
                            nc.tensor.transpose(o_ps2, oT_sb, ident_sb)
                            nc.scalar.copy(out=o_ext[:C, 128 * ds:128 * ds + 128], in_=o_ps2)

                        oh_ps = ps_vo.tile([C, 512], F32, tag="ohps")
                        nc.tensor.matmul(oh_ps, oT_w, o_ext, start=True, stop=True)
                        oh_sb = sbp.tile([C, 512], F32, tag="ohsb")
                        nc.vector.tensor_add(out=oh_sb, in0=oh_ps,
                                             in1=_fap(xkv_raw, C, s0 * sstep, [[sstep, 4], [pstep, 128]]))
                        dst = bass.AP(outs[f"outres_{br}"], b * C * PIX1 + 512 * g,
                                      [[PIX1, C], [1, 512]])
                        nc.sync.dma_start(out=dst, in_=oh_sb)
    return nc


# ================= kernel 2: 3x3 conv over concat(out_h, out_w) ============

@functools.lru_cache(maxsize=1)
def _build_k2():
    nc = bass.Bass()
    catp = nc.dram_tensor("catp", [B, 2 * C, CATR * CATW], F32, kind="ExternalInput")
    cwT = nc.dram_tensor("cwT", [2 * C, 9 * C], F32, kind="ExternalInput")
    cb = nc.dram_tensor("cb", [C, 1], F32, kind="ExternalInput")
    convout = nc.dram_tensor("convout", [B, C, ROWS3 * W], F32, kind="ExternalOutput")

    with tile.TileContext(nc) as tc:
        import contextlib
        ctx = contextlib.ExitStack()
        with ctx:
            const = ctx.enter_context(tc.tile_pool(name="const", bufs=1))
            sbp = ctx.enter_context(tc.tile_pool(name="sbp", bufs=2))
            psp = ctx.enter_context(tc.tile_pool(name="psp", bufs=4, space="PSUM"))

            cw_sb = const.tile([2 * C, 9 * C], F32)
            nc.sync.dma_start(out=cw_sb, in_=cwT[:, :])
            cb_sb = const.tile([C, 1], F32)
            nc.sync.dma_start(out=cb_sb, in_=cb[:, :])

            for b in range(B):
                x_sb = sbp.tile([2 * C, CATR * CATW], F32, tag="x")
                nc.sync.dma_start(out=x_sb, in_=catp[b, :, :])
                out_sb = sbp.tile([C, ROWS3 * W], F32, tag="o")
                for r0 in range(0, ROWS3, 4):
                    nr = min(4, ROWS3 - r0)
                    ps = psp.tile([C, 512], F32, tag="ps")
                    ti = 0
                    for dy in (-1, 0, 1):
                        for dx in (-1, 0, 1):
                            rhs = _fap(x_sb, 2 * C, (r0 + 1 + dy) * CATW + 1 + dx,
                                       [[CATW, nr], [1, W]])
                            nc.tensor.matmul(ps[:, :nr * W],
                                             cw_sb[:, (ti * C):(ti * C + C)],
                                             rhs, start=(ti == 0), stop=(ti == 8))
                            ti += 1
                    nc.vector.tensor_scalar_add(out=out_sb[:, r0 * W:(r0 + nr) * W],
                                                in0=ps[:, :nr * W], scalar1=cb_sb)
                nc.sync.dma_start(out=convout[b, :, :], in_=out_sb)
    return nc


# ================= kernel 3: LN3 + MLP =====================================

@functools.lru_cache(maxsize=1)
def _build_k3():
    nc = bass.Bass()
    convp = nc.dram_tensor("convp", [B, C, PIX3], F32, kind="ExternalInput")
    a3 = nc.dram_tensor("a3", [B, 1, PIX3], F32, kind="ExternalInput")
    b3 = nc.dram_tensor("b3", [B, 1, PIX3], F32, kind="ExternalInput")
    maskr = nc.dram_tensor("maskr", [1, PIX3], F32, kind="ExternalInput")
    w1T = nc.dram_tensor("w1T", [C + 1, HID], F32, kind="ExternalInput")
    dww = nc.dram_tensor("dww", [128, 2 * 84], F32, kind="ExternalInput")
    dwb = nc.dram_tensor("dwb", [128, 2 * 4], F32, kind="ExternalInput")
    w3T = nc.dram_tensor("w3T", [128, 8 * C], F32, kind="ExternalInput")
    b3c = nc.dram_tensor("b3c", [C, 1], F32, kind="ExternalInput")
    outp = nc.dram_tensor("outp", [B, C, OUTR * W], F32, kind="ExternalOutput")

    NPIX = OUTR * W
    taps = {}
    for k in (3, 5, 7):
        hw = k // 2
        taps[k] = [(dy, dx) for dy in range(-hw, hw + 1) for dx in range(-hw, hw + 1)]
    twoff = {1: 0, 3: 1, 5: 10, 7: 35}   # tap column offsets within 84

    with tile.TileContext(nc) as tc:
        import contextlib
        ctx = contextlib.ExitStack()
        with ctx:
            const = ctx.enter_context(tc.tile_pool(name="const", bufs=1))
            lnp = ctx.enter_context(tc.tile_pool(name="lnp", bufs=1))
            zp = ctx.enter_context(tc.tile_pool(name="zp", bufs=1))
            gp = ctx.enter_context(tc.tile_pool(name="gp", bufs=1))
            psp = ctx.enter_context(tc.tile_pool(name="psp", bufs=2, space="PSUM"))
            pso = ctx.enter_context(tc.tile_pool(name="pso", bufs=2, space="PSUM"))

            w1_sb = const.tile([C + 1, HID], F32)
            nc.sync.dma_start(out=w1_sb, in_=w1T[:, :])
            w3_sb = const.tile([128, 8 * C], F32)
            nc.sync.dma_start(out=w3_sb, in_=w3T[:, :])
            dww_sb = const.tile([128, 2 * 84], F32)
            nc.sync.dma_start(out=dww_sb, in_=dww[:, :])
            dwb_sb = const.tile([128, 2 * 4], F32)
            nc.sync.dma_start(out=dwb_sb, in_=dwb[:, :])
            b3c_sb = const.tile([C, 1], F32)
            nc.sync.dma_start(out=b3c_sb, in_=b3c[:, :])

            for b in range(B):
                x_sb = lnp.tile([C, PIX3], F32, tag="x")
                nc.sync.dma_start(out=x_sb, in_=convp[b, :, :])
                a_bc = lnp.tile([C, PIX3], F32, tag="abc")
                nc.sync.dma_start(out=a_bc, in_=_bc(a3, C, PIX3, b * PIX3))
                xn = lnp.tile([C + 1, PIX3], F32, tag="xn")
                nc.vector.tensor_mul(out=xn[:C, :], in0=x_sb, in1=a_bc)
                nc.gpsimd.dma_start(out=xn[:C, :], in_=_bc(b3, C, PIX3, b * PIX3), accum_op=AL.add)
                nc.sync.dma_start(out=xn[C:C + 1, :], in_=maskr[:, :])

                z = []
                for hf in range(2):
                    zt = zp.tile([128, PIX3], F32, tag=f"z{hf}")
                    for c0 in range(0, PIX3, 512):
                        nn_ = min(512, PIX3 - c0)
                        ps = psp.tile([128, 512], F32, tag="zps")
                        nc.tensor.matmul(ps[:, :nn_], w1_sb[:, 128 * hf:128 * hf + 128],
                                         xn[:, c0:c0 + nn_], start=True, stop=True)
                        nc.scalar.activation(out=zt[:, c0:c0 + nn_], in_=ps[:, :nn_],
                                             func=AF.Gelu)
                    z.append(zt)

                gtiles = []
                for ki, k in enumerate((1, 3, 5, 7)):
                    for hf in range(2):
                        gt = gp.tile([128, NPIX], F32, tag=f"g{ki}{hf}")
                        zt = z[hf]
                        wcol0 = 84 * hf + twoff[k]

                        def zin(dy, dx):
                            return _fap(zt, 128, (3 + dy) * W3PAD + 3 + dx,
                                        [[W3PAD, OUTR], [1, W]])

                        if k == 1:
                            nc.scalar.activation(out=gt, in_=zin(0, 0), func=AF.Gelu,
                                                 scale=dww_sb[:, wcol0:wcol0 + 1],
                                                 bias=dwb_sb[:, 4 * hf + ki:4 * hf + ki + 1])
                        else:
                            acc = gp.tile([128, NPIX], F32, tag="acc")
                            eng = nc.vector
                            for ti, (dy, dx) in enumerate(taps[k]):
                                wcol = dww_sb[:, wcol0 + ti:wcol0 + ti + 1]
                                if ti == 0:
                                    eng.tensor_scalar_mul(out=acc, in0=zin(dy, dx), scalar1=wcol)
                                else:
                                    eng.scalar_tensor_tensor(out=acc, in0=zin(dy, dx), scalar=wcol,
                                                             in1=acc, op0=AL.mult, op1=AL.add)
                            nc.scalar.activation(out=gt, in_=acc, func=AF.Gelu,
                                                 bias=dwb_sb[:, 4 * hf + ki:4 * hf + ki + 1],
                                                 scale=1.0)
                        gtiles.append(gt)
                # reorder gtiles from (ki, hf) pairs to K-chunk order:
                # gcat channels = [dw1(2 tiles), dw2(2), dw3(2), dw4(2)] already in order
                out_sb = gp.tile([C, NPIX], F32, tag="out")
                for c0 in range(0, NPIX, 512):
                    ps = pso.tile([C, 512], F32, tag="ops")
                    for kc in range(8):
                        nc.tensor.matmul(ps, w3_sb[:, C * kc:C * kc + C],
                                         gtiles[kc][:, c0:c0 + 512],
                                         start=(kc == 0), stop=(kc == 7))
                    res = _fap(x_sb, C, 3 * W3PAD + 3 + (c0 // W) * W3PAD, [[W3PAD, 4], [1, W]])
                    nc.vector.scalar_tensor_tensor(out=out_sb[:, c0:c0 + 512], in0=ps,
                                                   scalar=b3c_sb, in1=res,
                                                   op0=AL.add, op1=AL.add)
                nc.sync.dma_start(out=outp[b, :, :], in_=out_sb)
    return nc


# ======================= host orchestration ================================

def _ln_ab(x):
    mu = x.mean(axis=1)
    var = x.var(axis=1)
    rstd = 1.0 / np.sqrt(var + EPS)
    return rstd.astype(np.float32), (-mu * rstd).astype(np.float32)


def _wpack(wn, bn, g, bln):
    w_eff = wn * g[None, :]
    b_eff = wn @ bln + bn
    return np.ascontiguousarray(
        np.concatenate([w_eff.T, b_eff[None, :]], axis=0), dtype=np.float32)


def kernel(pan, lms, params):
    pan = np.asarray(pan, dtype=np.float32)
    lms = np.asarray(lms, dtype=np.float32)
    p = {k: (np.asarray(v, dtype=np.float32) if not isinstance(v, dict) else
             {k2: np.asarray(v2, dtype=np.float32) for k2, v2 in v.items()})
         for k, v in params.items()}

    g1, b1 = p["ln1_g"], p["ln1_b"]
    g2, b2 = p["ln2_g"], p["ln2_b"]
    a_pan, be_pan = _ln_ab(pan)
    a_lms, be_lms = _ln_ab(lms)
    ident = np.eye(128, dtype=np.float32)

    in_maps = []
    for cj in range(NC_):
        ws = SL * cj
        m = {"ident": ident}
        for br, ap_ in (("h", p["attn_h"]), ("w", p["attn_w"])):
            m[f"qT_{br}"] = _wpack(ap_["wq"], ap_["bq"], g1, b1)
            m[f"kT_{br}"] = _wpack(ap_["wk"], ap_["bk"], g2, b2)
            m[f"vT_{br}"] = _wpack(ap_["wv"], ap_["bv"], g2, b2)
            m[f"oT_{br}"] = np.ascontiguousarray(
                np.concatenate([ap_["wo"].T, ap_["bo"][None, :]], axis=0), dtype=np.float32)
        m["xq_h"] = np.ascontiguousarray(pan[:, :, :, ws:ws + SL]).reshape(B, C, PIX1)
        m["xkv_h"] = np.ascontiguousarray(lms[:, :, :, ws:ws + SL]).reshape(B, C, PIX1)
        m["aq_h"] = np.ascontiguousarray(a_pan[:, :, ws:ws + SL]).reshape(B, 1, PIX1)
        m["bq_h"] = np.ascontiguousarray(be_pan[:, :, ws:ws + SL]).reshape(B, 1, PIX1)
        m["akv_h"] = np.ascontiguousarray(a_lms[:, :, ws:ws + SL]).reshape(B, 1, PIX1)
        m["bkv_h"] = np.ascontiguousarray(be_lms[:, :, ws:ws + SL]).reshape(B, 1, PIX1)
        m["xq_w"] = np.ascontiguousarray(pan[:, :, ws:ws + SL, :]).reshape(B, C, PIX1)
        m["xkv_w"] = np.ascontiguousarray(lms[:, :, ws:ws + SL, :]).reshape(B, C, PIX1)
        m["aq_w"] = np.ascontiguousarray(a_pan[:, ws:ws + SL, :]).reshape(B, 1, PIX1)
        m["bq_w"] = np.ascontiguousarray(be_pan[:, ws:ws + SL, :]).reshape(B, 1, PIX1)
        m["akv_w"] = np.ascontiguousarray(a_lms[:, ws:ws + SL, :]).reshape(B, 1, PIX1)
        m["bkv_w"] = np.ascontiguousarray(be_lms[:, ws:ws + SL, :]).reshape(B, 1, PIX1)
        in_maps.append(m)

    r1 = run_bass_kernel_spmd(_build_k1(), in_maps, core_ids=list(range(NC_))).results

    out_h = np.empty((B, C, H, W), np.float32)
    out_w = np.empty((B, C, H, W), np.float32)
    amap_h = np.empty((B * W, HEADS, H, H), np.float32)
    amap_w = np.empty((B * H, HEADS, W, W), np.float32)
    ah5 = amap_h.reshape(B, NC_, SL, HEADS, H, H)
    aw5 = amap_w.reshape(B, NC_, SL, HEADS, W, W)
    for cj in range(NC_):
        ws = SL * cj
        res = r1[cj]
        out_h[:, :, :, ws:ws + SL] = res["outres_h"].reshape(B, C, SL, H).transpose(0, 1, 3, 2)
        out_w[:, :, ws:ws + SL, :] = res["outres_w"].reshape(B, C, SL, W)
        for br, dst in (("h", ah5), ("w", aw5)):
            e = res[f"eamap_{br}"]
            rc = res[f"rcp_{br}"]
            dst[:, cj] = e.transpose(0, 1, 2, 4, 3) * rc.transpose(0, 1, 3, 2)[:, :, :, :, None]

    cat = np.concatenate([out_h, out_w], axis=1)
    catp_full = np.zeros((B, 2 * C, H + 8, CATW), np.float32)
    catp_full[:, :, 4:4 + H, 1:1 + W] = cat
    cw = p["conv_w"]
    cwT = np.ascontiguousarray(cw.transpose(1, 2, 3, 0).reshape(2 * C, 9 * C))
    in_maps2 = []
    for cj in range(NC_):
        r0 = SL * cj
        in_maps2.append({
            "catp": np.ascontiguousarray(catp_full[:, :, r0:r0 + CATR, :]).reshape(B, 2 * C, CATR * CATW),
            "cwT": cwT,
            "cb": np.ascontiguousarray(p["conv_b"][:, None]),
        })
    r2 = run_bass_kernel_spmd(_build_k2(), in_maps2, core_ids=list(range(NC_))).results

    convout = np.empty((B, C, H, W), np.float32)
    for cj in range(NC_):
        co = r2[cj]["convout"].reshape(B, C, ROWS3, W)
        convout[:, :, SL * cj:SL * cj + SL, :] = co[:, :, 3:3 + SL, :]

    g3, bl3 = p["ln3_g"], p["ln3_b"]
    a3f, b3f = _ln_ab(convout)
    mp = p["mlp"]
    w1T = np.ascontiguousarray(np.concatenate(
        [(mp["w1"] * g3[None, :]).T, (mp["w1"] @ bl3 + mp["b1"])[None, :]], axis=0),
        dtype=np.float32)
    dww = np.concatenate([mp["dw1"].reshape(HID, 1), mp["dw2"].reshape(HID, 9),
                          mp["dw3"].reshape(HID, 25), mp["dw4"].reshape(HID, 49)],
                         axis=1).astype(np.float32)
    dww_r = np.ascontiguousarray(dww.reshape(2, 128, 84).transpose(1, 0, 2).reshape(128, 2 * 84))
    dwb = np.stack([mp["db1"], mp["db2"], mp["db3"], mp["db4"]], axis=1).astype(np.float32)
    dwb_r = np.ascontiguousarray(dwb.reshape(2, 128, 4).transpose(1, 0, 2).reshape(128, 2 * 4))
    w3T_r = np.ascontiguousarray(mp["w3"].T.reshape(8, 128, C).transpose(1, 0, 2).reshape(128, 8 * C))

    a3p_full = np.zeros((B, H + 6, W3PAD), np.float32)
    b3p_full = np.zeros((B, H + 6, W3PAD), np.float32)
    a3p_full[:, 3:3 + H, 3:3 + W] = a3f
    b3p_full[:, 3:3 + H, 3:3 + W] = b3f
    convp_full = np.zeros((B, C, H + 6, W3PAD), np.float32)
    convp_full[:, :, 3:3 + H, 3:3 + W] = convout
    mask_full = np.zeros((H + 6, W3PAD), np.float32)
    mask_full[3:3 + H, 3:3 + W] = 1.0

    in_maps3 = []
    for cj in range(NC_):
        r0 = SL * cj
        in_maps3.append({
            "convp": np.ascontiguousarray(convp_full[:, :, r0:r0 + ROWS3, :]).reshape(B, C, PIX3),
            "a3": np.ascontiguousarray(a3p_full[:, r0:r0 + ROWS3, :]).reshape(B, 1, PIX3),
            "b3": np.ascontiguousarray(b3p_full[:, r0:r0 + ROWS3, :]).reshape(B, 1, PIX3),
            "maskr": np.ascontiguousarray(mask_full[r0:r0 + ROWS3, :]).reshape(1, PIX3),
            "w1T": w1T, "dww": dww_r, "dwb": dwb_r, "w3T": w3T_r,
            "b3c": np.ascontiguousarray(mp["b3"][:, None]),
        })
    r3 = run_bass_kernel_spmd(_build_k3(), in_maps3, core_ids=list(range(NC_))).results

    output = np.empty((B, C, H, W), np.float32)
    for cj in range(NC_):
        output[:, :, SL * cj:SL * cj + SL, :] = r3[cj]["outp"].reshape(B, C, OUTR, W)

    return output, amap_h, amap_w


# revision 9
# speedup vs baseline: 1.3626x; 1.3626x over previous
"""Trainium2 Bass kernel for nn_Block1 (axial cross-attention block).

Sharding: pure data-parallel across 8 NeuronCores.
  - attn_h: each core takes a 16-wide w-range (all b, full h).
  - attn_w: each core takes a 16-tall h-range (all b, full w).
  - conv3x3 + MLP: each core takes a 16-tall h-range of output rows.
Three NEFF launches with host-side resharding between them.

Device kernels keep softmax un-normalized: exp(scores^T) [j,i] tiles are
DMA'd out directly plus per-row reciprocals; the host does amap = exp.T * rcp.
LayerNorm stats (rstd/-mu*rstd per pixel) are computed host-side (a tiny
reduction); the apply (x*alpha + beta) runs on-device (DVE mult + DMA
accumulate-add). LN gamma/beta are folded into consuming conv weights.
"""

import functools
import numpy as np

import concourse.bass as bass
import concourse.tile as tile
from concourse import mybir
from concourse.bass_utils import run_bass_kernel_spmd

# ---- walrus sync-wait workaround ----
import bass_rust
from concourse.vector_clock import ScopedClock

_MAXW = 1
_patch_state = {"done": False, "ctr": 0}


def _mk_nop(engine, wait):
    nop = bass_rust.InstNoOp(name=f"I-waitsplit-{_patch_state['ctr']}", ins=[], outs=[])
    _patch_state["ctr"] += 1
    nop.engine = engine
    nop.sync_info = bass_rust.SyncInfo(on_wait=[wait], on_update=[])
    return nop


def _split_list(insts):
    out = []
    changed = False
    for inst in insts:
        si = getattr(inst, "sync_info", None)
        waits = list(si.on_wait) if (si is not None and si.on_wait) else []
        if len(waits) > _MAXW:
            for w in waits[:-_MAXW]:
                out.append(_mk_nop(inst.engine, w))
            si.on_wait = waits[-_MAXW:]
            changed = True
        out.append(inst)
    if changed:
        insts[:] = out


def _patch_tile():
    if _patch_state["done"]:
        return
    _patch_state["done"] = True

    orig_lower = tile.TileContext._lower_ordered_insts

    def _lower_ordered_insts(self, ordered):
        for insts in ordered.values():
            _split_list(insts)
        return orig_lower(self, ordered)

    tile.TileContext._lower_ordered_insts = _lower_ordered_insts

    def _drain_and_barrier(self, tick_clock, wait_clock):
        drain_inst = self.nc.sync.drain()
        wait_clock.add_sem_waits(
            drain_inst.ins, ScopedClock({None: tick_clock.global_clock})
        )
        si = drain_inst.ins.sync_info
        waits = list(si.on_wait or [])
        if len(waits) > 1:
            si.on_wait = waits[:1]
            for w in waits[1:]:
                d2 = self.nc.sync.drain()
                si2 = d2.ins.sync_info
                if si2 is None:
                    d2.ins.sync_info = bass_rust.SyncInfo(on_wait=[w], on_update=[])
                else:
                    si2.on_wait = [w]

        self.nc.all_engine_barrier()
        assert self.sems is not None
        popped = self.nc._tile_sem_poison_stack.pop()
        assert popped is self._sem_poison
        self.nc.clear_and_free_semaphores(list(self.sems.allocated().values()))
        self.nc.all_engine_barrier()

    tile.TileContext._drain_and_barrier = _drain_and_barrier


_patch_tile()


def _install_neff_cache():
    """Disk-cache walrus NEFF compiles keyed on the BIR json hash."""
    import hashlib
    import os
    import shutil
    from concourse import bass2jax

    if getattr(bass2jax, "_neff_cache_installed", False):
        return
    bass2jax._neff_cache_installed = True
    orig = bass2jax.compile_bir_kernel
    cache_dir = os.path.expanduser("~/.bass_neff_cache")
    os.makedirs(cache_dir, exist_ok=True)

    def cached(bir_json, tmpdir, neff_name="file.neff"):
        key = hashlib.sha256(bir_json).hexdigest()
        p = os.path.join(cache_dir, key + ".neff")
        tgt = os.path.join(tmpdir, neff_name)
        if os.path.exists(p):
            shutil.copy(p, tgt)
            return tgt
        f = orig(bir_json, tmpdir, neff_name=neff_name)
        try:
            shutil.copy(f, p)
        except OSError:
            pass
        return f

    bass2jax.compile_bir_kernel = cached


_install_neff_cache()

# ---- problem constants ----
B, C, H, W = 4, 64, 128, 128
HEADS, DH = 8, 8
HID = 256
NC_ = 8
SL = 16
EPS = 1e-5
SCALE = C ** -0.5
F32 = mybir.dt.float32
BF16 = mybir.dt.bfloat16
AL = mybir.AluOpType
AF = mybir.ActivationFunctionType

PIX1 = H * SL              # 2048
ROWS3 = 22
W3PAD = W + 6              # 134
PIX3 = ROWS3 * W3PAD       # 2948
OUTR = 16
CATR = 24
CATW = 130


def _bc(t, parts, n, offset=0):
    return bass.AP(t, offset, [[0, parts], [1, n]])


def _fap(tile_t, nparts, offset, dims):
    """AP into a pool tile: partitions [0, nparts), free dims given explicitly."""
    a = tile_t[:nparts, :]
    return bass.AP(a.tensor, a.offset + offset, [a.ap[0]] + dims)


def _pslice(tile_t, p0, np_, offset, dims):
    """AP into tile partitions [p0, p0+np_) with explicit free dims."""
    a = tile_t[p0:p0 + np_, :]
    return bass.AP(a.tensor, a.offset + offset, [a.ap[0]] + dims)


# ================= kernel 1: layernorms + both attention branches ==========

@functools.lru_cache(maxsize=1)
def _build_k1():
    nc = bass.Bass()

    ins = {}
    for br in ("h", "w"):
        ins[f"xq_{br}"] = nc.dram_tensor(f"xq_{br}", [B, C, PIX1], F32, kind="ExternalInput")
        ins[f"xkv_{br}"] = nc.dram_tensor(f"xkv_{br}", [B, C, PIX1], F32, kind="ExternalInput")
        for nm in ("aq", "bq", "akv", "bkv"):
            ins[f"{nm}_{br}"] = nc.dram_tensor(f"{nm}_{br}", [B, 1, PIX1], F32, kind="ExternalInput")
        for nm in ("qTs", "kTs"):
            ins[f"{nm}_{br}"] = nc.dram_tensor(f"{nm}_{br}", [C + 1, 2 * 128], BF16, kind="ExternalInput")
        for nm in ("vT", "oT"):
            ins[f"{nm}_{br}"] = nc.dram_tensor(f"{nm}_{br}", [C + 1, C], BF16, kind="ExternalInput")
    ident = nc.dram_tensor("ident", [128, 128], F32, kind="ExternalInput")

    outs = {}
    for br in ("h", "w"):
        outs[f"eamap_{br}"] = nc.dram_tensor(f"eamap_{br}", [B, SL, HEADS, 128, 128], BF16, kind="ExternalOutput")
        outs[f"rcp_{br}"] = nc.dram_tensor(f"rcp_{br}", [B, SL, 128, HEADS], F32, kind="ExternalOutput")
        outs[f"outres_{br}"] = nc.dram_tensor(f"outres_{br}", [B, C, PIX1], F32, kind="ExternalOutput")

    with tile.TileContext(nc) as tc:
        import contextlib
        ctx = contextlib.ExitStack()
        with ctx:
            const = ctx.enter_context(tc.tile_pool(name="const", bufs=1))
            lnp = ctx.enter_context(tc.tile_pool(name="lnp", bufs=2))
            sbp = ctx.enter_context(tc.tile_pool(name="sbp", bufs=3))
            expp = ctx.enter_context(tc.tile_pool(name="expp", bufs=4))
            ps_qk = ctx.enter_context(tc.tile_pool(name="ps_qk", bufs=2, space="PSUM"))
            ps_vo = ctx.enter_context(tc.tile_pool(name="ps_vo", bufs=1, space="PSUM"))
            ps_sT = ctx.enter_context(tc.tile_pool(name="ps_sT", bufs=2, space="PSUM"))
            ps_sm = ctx.enter_context(tc.tile_pool(name="ps_sm", bufs=1, space="PSUM"))
            ps_oa = ctx.enter_context(tc.tile_pool(name="ps_oa", bufs=1, space="PSUM"))

            ident_sb = const.tile([128, 128], F32)
            nc.sync.dma_start(out=ident_sb, in_=ident[:, :])

            wsb = {}
            for br in ("h", "w"):
                for nm in ("qTs", "kTs"):
                    t = const.tile([C + 1, 2 * 128], BF16, tag=f"w_{nm}_{br}")
                    nc.sync.dma_start(out=t, in_=ins[f"{nm}_{br}"][:, :])
                    wsb[f"{nm}_{br}"] = t
                for nm in ("vT", "oT"):
                    t = const.tile([C + 1, C], BF16, tag=f"w_{nm}_{br}")
                    nc.sync.dma_start(out=t, in_=ins[f"{nm}_{br}"][:, :])
                    wsb[f"{nm}_{br}"] = t

            for b in range(B):
                for br in ("h", "w"):
                    if br == "h":
                        pstep, sstep = SL, 1      # col = 16*pix + s
                    else:
                        pstep, sstep = 1, 128     # col = 128*s + pix

                    # ---- LN applies ----
                    xq_raw = lnp.tile([C, PIX1], F32, tag="xq_raw")
                    nc.sync.dma_start(out=xq_raw, in_=ins[f"xq_{br}"][b, :, :])
                    a_bc = lnp.tile([C, PIX1], F32, tag="a_bc")
                    nc.sync.dma_start(out=a_bc, in_=_bc(ins[f"aq_{br}"], C, PIX1, b * PIX1))
                    xnq = lnp.tile([C + 1, PIX1], BF16, tag="xnq")
                    nc.vector.tensor_mul(out=xnq[:C, :], in0=xq_raw, in1=a_bc)
                    nc.gpsimd.dma_start(out=xnq[:C, :], in_=_bc(ins[f"bq_{br}"], C, PIX1, b * PIX1), accum_op=AL.add)
                    nc.vector.memset(xnq[C:C + 1, :], 1.0)

                    xkv_raw = lnp.tile([C, PIX1], F32, tag="xkv_raw")
                    nc.sync.dma_start(out=xkv_raw, in_=ins[f"xkv_{br}"][b, :, :])
                    akv_bc = lnp.tile([C, PIX1], F32, tag="akv_bc")
                    nc.sync.dma_start(out=akv_bc, in_=_bc(ins[f"akv_{br}"], C, PIX1, b * PIX1))
                    xnkv = lnp.tile([C + 1, PIX1], BF16, tag="xnkv")
                    nc.vector.tensor_mul(out=xnkv[:C, :], in0=xkv_raw, in1=akv_bc)
                    nc.gpsimd.dma_start(out=xnkv[:C, :], in_=_bc(ins[f"bkv_{br}"], C, PIX1, b * PIX1), accum_op=AL.add)
                    nc.vector.memset(xnkv[C:C + 1, :], 1.0)

                    qT_w, kT_w, vT_w, oT_w = (wsb[f"{nm}_{br}"] for nm in ("qTs", "kTs", "vT", "oT"))

                    for g in range(SL // 4):
                        s0 = 4 * g
                        rhs_q = _fap(xnq, C + 1, s0 * sstep, [[sstep, 4], [pstep, 128]])
                        rhs_kv = _fap(xnkv, C + 1, s0 * sstep, [[sstep, 4], [pstep, 128]])

                        qk_sb = {}
                        for nm, rhs, wT in (("q", rhs_q, qT_w), ("k", rhs_kv, kT_w)):
                            for half in range(2):
                                ps = ps_qk.tile([128, 512], F32, tag="qkspread")
                                nc.tensor.matmul(ps, wT[:, 128 * half:128 * half + 128],
                                                 rhs, start=True, stop=True)
                                sb = sbp.tile([128, 512], BF16, tag=f"{nm}sb{half}")
                                if nm == "q":
                                    nc.scalar.copy(out=sb, in_=ps)
                                else:
                                    nc.vector.tensor_copy(out=sb, in_=ps)
                                qk_sb[(nm, half)] = sb

                        v_ps = ps_vo.tile([C, 512], F32, tag="vps")
                        nc.tensor.matmul(v_ps, vT_w, rhs_kv, start=True, stop=True)
                        v_sb = sbp.tile([C, 512], F32, tag="vsb")
                        nc.vector.tensor_copy(out=v_sb, in_=v_ps)

                        o_ext = sbp.tile([C + 1, 512], BF16, tag="oext")
                        nc.vector.memset(o_ext[C:C + 1, :], 1.0)

                        for ds in range(4):
                            s = s0 + ds
                            vT_ps = ps_sm.tile([128, C], F32, tag="small")
                            nc.tensor.transpose(vT_ps, _fap(v_sb, C, 128 * ds, [[1, 128]]), ident_sb[:C, :C])
                            vT_ext = sbp.tile([128, 9 * HEADS], BF16, tag="vText")
                            nc.vector.memset(_fap(vT_ext, 128, 8, [[9, 8]]), 1.0)
                            nc.vector.tensor_copy(out=_fap(vT_ext, 128, 0, [[9, 8], [1, 8]]), in_=vT_ps)

                            o_all = ps_oa.tile([128, 9 * HEADS], F32, tag="oall")
                            for n_head in range(HEADS):
                                half, m = divmod(n_head, 4)
                                lhs = _pslice(qk_sb[("k", half)], 32 * m, 8, 128 * ds, [[1, 128]])
                                rhs = _pslice(qk_sb[("q", half)], 32 * m, 8, 128 * ds, [[1, 128]])
                                sT_ps = ps_sT.tile([128, 128], F32, tag="sT")
                                nc.tensor.matmul(sT_ps, lhs, rhs, start=True, stop=True,
                                                 tile_position=(32 * m, 0))
                                expT = expp.tile([128, 128], BF16, tag="expT")
                                nc.scalar.activation(out=expT, in_=sT_ps, func=AF.Exp, scale=SCALE)
                                nc.sync.dma_start(out=outs[f"eamap_{br}"][b, s, n_head, :, :], in_=expT)
                                nc.tensor.matmul(o_all[:, 9 * n_head:9 * n_head + 9], expT,
                                                 vT_ext[:, 9 * n_head:9 * n_head + 9],
                                                 start=True, stop=True)

                            rcp_sb = sbp.tile([128, HEADS], F32, tag="rcp")
                            nc.vector.reciprocal(out=rcp_sb, in_=_fap(o_all, 128, 8, [[9, 8]]))
                            nc.sync.dma_start(out=outs[f"rcp_{br}"][b, s, :, :], in_=rcp_sb)

                            oT_sb = sbp.tile([128, C], F32, tag="oTsb")
                            nc.vector.tensor_mul(
                                out=_fap(oT_sb, 128, 0, [[8, 8], [1, 8]]),
                                in0=_fap(o_all, 128, 0, [[9, 8], [1, 8]]),
                                in1=_fap(rcp_sb, 128, 0, [[1, 8], [0, 8]]),
                            )

                            o_ps2 = ps_sm.tile([C, 128], F32, tag="small")
                            nc.tensor.transpose(o_ps2, oT_sb, ident_sb)
                            nc.scalar.copy(out=o_ext[:C, 128 * ds:128 * ds + 128], in_=o_ps2)

                        oh_ps = ps_vo.tile([C, 512], F32, tag="ohps")
                        nc.tensor.matmul(oh_ps, oT_w, o_ext, start=True, stop=True)
                        oh_sb = sbp.tile([C, 512], F32, tag="ohsb")
                        nc.vector.tensor_add(out=oh_sb, in0=oh_ps,
                                             in1=_fap(xkv_raw, C, s0 * sstep, [[sstep, 4], [pstep, 128]]))
                        dst = bass.AP(outs[f"outres_{br}"], b * C * PIX1 + 512 * g,
                                      [[PIX1, C], [1, 512]])
                        nc.sync.dma_start(out=dst, in_=oh_sb)
    return nc


# ================= kernel 2: 3x3 conv over concat(out_h, out_w) ============

@functools.lru_cache(maxsize=1)
def _build_k2():
    nc = bass.Bass()
    catp = nc.dram_tensor("catp", [B, 2 * C, CATR * CATW], F32, kind="ExternalInput")
    cwT = nc.dram_tensor("cwT", [2 * C, 9 * C], F32, kind="ExternalInput")
    cb = nc.dram_tensor("cb", [C, 1], F32, kind="ExternalInput")
    convout = nc.dram_tensor("convout", [B, C, ROWS3 * W], F32, kind="ExternalOutput")

    with tile.TileContext(nc) as tc:
        import contextlib
        ctx = contextlib.ExitStack()
        with ctx:
            const = ctx.enter_context(tc.tile_pool(name="const", bufs=1))
            sbp = ctx.enter_context(tc.tile_pool(name="sbp", bufs=2))
            psp = ctx.enter_context(tc.tile_pool(name="psp", bufs=4, space="PSUM"))

            cw_sb = const.tile([2 * C, 9 * C], F32)
            nc.sync.dma_start(out=cw_sb, in_=cwT[:, :])
            cb_sb = const.tile([C, 1], F32)
            nc.sync.dma_start(out=cb_sb, in_=cb[:, :])

            for b in range(B):
                x_sb = sbp.tile([2 * C, CATR * CATW], F32, tag="x")
                nc.sync.dma_start(out=x_sb, in_=catp[b, :, :])
                out_sb = sbp.tile([C, ROWS3 * W], F32, tag="o")
                for r0 in range(0, ROWS3, 4):
                    nr = min(4, ROWS3 - r0)
                    ps = psp.tile([C, 512], F32, tag="ps")
                    ti = 0
                    for dy in (-1, 0, 1):
                        for dx in (-1, 0, 1):
                            rhs = _fap(x_sb, 2 * C, (r0 + 1 + dy) * CATW + 1 + dx,
                                       [[CATW, nr], [1, W]])
                            nc.tensor.matmul(ps[:, :nr * W],
                                             cw_sb[:, (ti * C):(ti * C + C)],
                                             rhs, start=(ti == 0), stop=(ti == 8))
                            ti += 1
                    nc.vector.tensor_scalar_add(out=out_sb[:, r0 * W:(r0 + nr) * W],
                                                in0=ps[:, :nr * W], scalar1=cb_sb)
                nc.sync.dma_start(out=convout[b, :, :], in_=out_sb)
    return nc


# ================= kernel 3: LN3 + MLP =====================================

@functools.lru_cache(maxsize=1)
def _build_k3():
    nc = bass.Bass()
    convp = nc.dram_tensor("convp", [B, C, PIX3], F32, kind="ExternalInput")
    a3 = nc.dram_tensor("a3", [B, 1, PIX3], F32, kind="ExternalInput")
    b3 = nc.dram_tensor("b3", [B, 1, PIX3], F32, kind="ExternalInput")
    maskr = nc.dram_tensor("maskr", [1, PIX3], F32, kind="ExternalInput")
    w1T = nc.dram_tensor("w1T", [C + 1, HID], F32, kind="ExternalInput")
    dww = nc.dram_tensor("dww", [128, 2 * 84], F32, kind="ExternalInput")
    dwb = nc.dram_tensor("dwb", [128, 2 * 4], F32, kind="ExternalInput")
    w3T = nc.dram_tensor("w3T", [128, 8 * C], F32, kind="ExternalInput")
    b3c = nc.dram_tensor("b3c", [C, 1], F32, kind="ExternalInput")
    outp = nc.dram_tensor("outp", [B, C, OUTR * W], F32, kind="ExternalOutput")

    NPIX = OUTR * W
    taps = {}
    for k in (3, 5, 7):
        hw = k // 2
        taps[k] = [(dy, dx) for dy in range(-hw, hw + 1) for dx in range(-hw, hw + 1)]
    twoff = {1: 0, 3: 1, 5: 10, 7: 35}   # tap column offsets within 84

    with tile.TileContext(nc) as tc:
        import contextlib
        ctx = contextlib.ExitStack()
        with ctx:
            const = ctx.enter_context(tc.tile_pool(name="const", bufs=1))
            lnp = ctx.enter_context(tc.tile_pool(name="lnp", bufs=1))
            zp = ctx.enter_context(tc.tile_pool(name="zp", bufs=1))
            gp = ctx.enter_context(tc.tile_pool(name="gp", bufs=1))
            psp = ctx.enter_context(tc.tile_pool(name="psp", bufs=2, space="PSUM"))
            pso = ctx.enter_context(tc.tile_pool(name="pso", bufs=2, space="PSUM"))

            w1_sb = const.tile([C + 1, HID], F32)
            nc.sync.dma_start(out=w1_sb, in_=w1T[:, :])
            w3_sb = const.tile([128, 8 * C], F32)
            nc.sync.dma_start(out=w3_sb, in_=w3T[:, :])
            dww_sb = const.tile([128, 2 * 84], F32)
            nc.sync.dma_start(out=dww_sb, in_=dww[:, :])
            dwb_sb = const.tile([128, 2 * 4], F32)
            nc.sync.dma_start(out=dwb_sb, in_=dwb[:, :])
            b3c_sb = const.tile([C, 1], F32)
            nc.sync.dma_start(out=b3c_sb, in_=b3c[:, :])

            for b in range(B):
                x_sb = lnp.tile([C, PIX3], F32, tag="x")
                nc.sync.dma_start(out=x_sb, in_=convp[b, :, :])
                a_bc = lnp.tile([C, PIX3], F32, tag="abc")
                nc.sync.dma_start(out=a_bc, in_=_bc(a3, C, PIX3, b * PIX3))
                xn = lnp.tile([C + 1, PIX3], F32, tag="xn")
                nc.vector.tensor_mul(out=xn[:C, :], in0=x_sb, in1=a_bc)
                nc.gpsimd.dma_start(out=xn[:C, :], in_=_bc(b3, C, PIX3, b * PIX3), accum_op=AL.add)
                nc.sync.dma_start(out=xn[C:C + 1, :], in_=maskr[:, :])

                z = []
                for hf in range(2):
                    zt = zp.tile([128, PIX3], F32, tag=f"z{hf}")
                    for c0 in range(0, PIX3, 512):
                        nn_ = min(512, PIX3 - c0)
                        ps = psp.tile([128, 512], F32, tag="zps")
                        nc.tensor.matmul(ps[:, :nn_], w1_sb[:, 128 * hf:128 * hf + 128],
                                         xn[:, c0:c0 + nn_], start=True, stop=True)
                        nc.scalar.activation(out=zt[:, c0:c0 + nn_], in_=ps[:, :nn_],
                                             func=AF.Gelu)
                    z.append(zt)

                gtiles = []
                for ki, k in enumerate((1, 3, 5, 7)):
                    for hf in range(2):
                        gt = gp.tile([128, NPIX], F32, tag=f"g{ki}{hf}")
                        zt = z[hf]
                        wcol0 = 84 * hf + twoff[k]

                        def zin(dy, dx):
                            return _fap(zt, 128, (3 + dy) * W3PAD + 3 + dx,
                                        [[W3PAD, OUTR], [1, W]])

                        if k == 1:
                            nc.scalar.activation(out=gt, in_=zin(0, 0), func=AF.Gelu,
                                                 scale=dww_sb[:, wcol0:wcol0 + 1],
                                                 bias=dwb_sb[:, 4 * hf + ki:4 * hf + ki + 1])
                        else:
                            acc = gp.tile([128, NPIX], F32, tag="acc")
                            eng = nc.vector
                            for ti, (dy, dx) in enumerate(taps[k]):
                                wcol = dww_sb[:, wcol0 + ti:wcol0 + ti + 1]
                                if ti == 0:
                                    eng.tensor_scalar_mul(out=acc, in0=zin(dy, dx), scalar1=wcol)
                                else:
                                    eng.scalar_tensor_tensor(out=acc, in0=zin(dy, dx), scalar=wcol,
                                                             in1=acc, op0=AL.mult, op1=AL.add)
                            nc.scalar.activation(out=gt, in_=acc, func=AF.Gelu,
                                                 bias=dwb_sb[:, 4 * hf + ki:4 * hf + ki + 1],
                                                 scale=1.0)
                        gtiles.append(gt)
                # reorder gtiles from (ki, hf) pairs to K-chunk order:
                # gcat channels = [dw1(2 tiles), dw2(2), dw3(2), dw4(2)] already in order
                out_sb = gp.tile([C, NPIX], F32, tag="out")
                for c0 in range(0, NPIX, 512):
                    ps = pso.tile([C, 512], F32, tag="ops")
                    for kc in range(8):
                        nc.tensor.matmul(ps, w3_sb[:, C * kc:C * kc + C],
                                         gtiles[kc][:, c0:c0 + 512],
                                         start=(kc == 0), stop=(kc == 7))
                    res = _fap(x_sb, C, 3 * W3PAD + 3 + (c0 // W) * W3PAD, [[W3PAD, 4], [1, W]])
                    nc.vector.scalar_tensor_tensor(out=out_sb[:, c0:c0 + 512], in0=ps,
                                                   scalar=b3c_sb, in1=res,
                                                   op0=AL.add, op1=AL.add)
                nc.sync.dma_start(out=outp[b, :, :], in_=out_sb)
    return nc


# ======================= host orchestration ================================

def _ln_ab(x):
    mu = x.mean(axis=1)
    var = x.var(axis=1)
    rstd = 1.0 / np.sqrt(var + EPS)
    return rstd.astype(np.float32), (-mu * rstd).astype(np.float32)


BF16NP = mybir.dt.np(mybir.dt.bfloat16)


def _wpack_spread(wn, bn, g, bln):
    """[C+1, 2*128] bf16: half h, col 32m+d = (head 4h+m, dim d); zeros else."""
    w_eff = wn * g[None, :]
    b_eff = wn @ bln + bn
    full = np.concatenate([w_eff.T, b_eff[None, :]], axis=0)  # [65, 64]
    out = np.zeros((C + 1, 2 * 128), np.float32)
    for n in range(HEADS):
        half, m = divmod(n, 4)
        out[:, 128 * half + 32 * m:128 * half + 32 * m + DH] = full[:, DH * n:DH * n + DH]
    return np.ascontiguousarray(out.astype(BF16NP))


def _wpack(wn, bn, g, bln):
    w_eff = wn * g[None, :]
    b_eff = wn @ bln + bn
    return np.ascontiguousarray(
        np.concatenate([w_eff.T, b_eff[None, :]], axis=0), dtype=np.float32)


def kernel(pan, lms, params):
    pan = np.asarray(pan, dtype=np.float32)
    lms = np.asarray(lms, dtype=np.float32)
    p = {k: (np.asarray(v, dtype=np.float32) if not isinstance(v, dict) else
             {k2: np.asarray(v2, dtype=np.float32) for k2, v2 in v.items()})
         for k, v in params.items()}

    g1, b1 = p["ln1_g"], p["ln1_b"]
    g2, b2 = p["ln2_g"], p["ln2_b"]
    a_pan, be_pan = _ln_ab(pan)
    a_lms, be_lms = _ln_ab(lms)
    ident = np.eye(128, dtype=np.float32)

    in_maps = []
    for cj in range(NC_):
        ws = SL * cj
        m = {"ident": ident}
        for br, ap_ in (("h", p["attn_h"]), ("w", p["attn_w"])):
            m[f"qTs_{br}"] = _wpack_spread(ap_["wq"], ap_["bq"], g1, b1)
            m[f"kTs_{br}"] = _wpack_spread(ap_["wk"], ap_["bk"], g2, b2)
            m[f"vT_{br}"] = _wpack(ap_["wv"], ap_["bv"], g2, b2).astype(BF16NP)
            m[f"oT_{br}"] = np.ascontiguousarray(
                np.concatenate([ap_["wo"].T, ap_["bo"][None, :]], axis=0)).astype(BF16NP)
        m["xq_h"] = np.ascontiguousarray(pan[:, :, :, ws:ws + SL]).reshape(B, C, PIX1)
        m["xkv_h"] = np.ascontiguousarray(lms[:, :, :, ws:ws + SL]).reshape(B, C, PIX1)
        m["aq_h"] = np.ascontiguousarray(a_pan[:, :, ws:ws + SL]).reshape(B, 1, PIX1)
        m["bq_h"] = np.ascontiguousarray(be_pan[:, :, ws:ws + SL]).reshape(B, 1, PIX1)
        m["akv_h"] = np.ascontiguousarray(a_lms[:, :, ws:ws + SL]).reshape(B, 1, PIX1)
        m["bkv_h"] = np.ascontiguousarray(be_lms[:, :, ws:ws + SL]).reshape(B, 1, PIX1)
        m["xq_w"] = np.ascontiguousarray(pan[:, :, ws:ws + SL, :]).reshape(B, C, PIX1)
        m["xkv_w"] = np.ascontiguousarray(lms[:, :, ws:ws + SL, :]).reshape(B, C, PIX1)
        m["aq_w"] = np.ascontiguousarray(a_pan[:, ws:ws + SL, :]).reshape(B, 1, PIX1)
        m["bq_w"] = np.ascontiguousarray(be_pan[:, ws:ws + SL, :]).reshape(B, 1, PIX1)
        m["akv_w"] = np.ascontiguousarray(a_lms[:, ws:ws + SL, :]).reshape(B, 1, PIX1)
        m["bkv_w"] = np.ascontiguousarray(be_lms[:, ws:ws + SL, :]).reshape(B, 1, PIX1)
        in_maps.append(m)

    r1 = run_bass_kernel_spmd(_build_k1(), in_maps, core_ids=list(range(NC_))).results

    out_h = np.empty((B, C, H, W), np.float32)
    out_w = np.empty((B, C, H, W), np.float32)
    amap_h = np.empty((B * W, HEADS, H, H), np.float32)
    amap_w = np.empty((B * H, HEADS, W, W), np.float32)
    ah5 = amap_h.reshape(B, NC_, SL, HEADS, H, H)
    aw5 = amap_w.reshape(B, NC_, SL, HEADS, W, W)
    for cj in range(NC_):
        ws = SL * cj
        res = r1[cj]
        out_h[:, :, :, ws:ws + SL] = res["outres_h"].reshape(B, C, SL, H).transpose(0, 1, 3, 2)
        out_w[:, :, ws:ws + SL, :] = res["outres_w"].reshape(B, C, SL, W)
        for br, dst in (("h", ah5), ("w", aw5)):
            e = res[f"eamap_{br}"].astype(np.float32)
            rc = res[f"rcp_{br}"]
            dst[:, cj] = e.transpose(0, 1, 2, 4, 3) * rc.transpose(0, 1, 3, 2)[:, :, :, :, None]

    cat = np.concatenate([out_h, out_w], axis=1)
    catp_full = np.zeros((B, 2 * C, H + 8, CATW), np.float32)
    catp_full[:, :, 4:4 + H, 1:1 + W] = cat
    cw = p["conv_w"]
    cwT = np.ascontiguousarray(cw.transpose(1, 2, 3, 0).reshape(2 * C, 9 * C))
    in_maps2 = []
    for cj in range(NC_):
        r0 = SL * cj
        in_maps2.append({
            "catp": np.ascontiguousarray(catp_full[:, :, r0:r0 + CATR, :]).reshape(B, 2 * C, CATR * CATW),
            "cwT": cwT,
            "cb": np.ascontiguousarray(p["conv_b"][:, None]),
        })
    r2 = run_bass_kernel_spmd(_build_k2(), in_maps2, core_ids=list(range(NC_))).results

    convout = np.empty((B, C, H, W), np.float32)
    for cj in range(NC_):
        co = r2[cj]["convout"].reshape(B, C, ROWS3, W)
        convout[:, :, SL * cj:SL * cj + SL, :] = co[:, :, 3:3 + SL, :]

    g3, bl3 = p["ln3_g"], p["ln3_b"]
    a3f, b3f = _ln_ab(convout)
    mp = p["mlp"]
    w1T = np.ascontiguousarray(np.concatenate(
        [(mp["w1"] * g3[None, :]).T, (mp["w1"] @ bl3 + mp["b1"])[None, :]], axis=0),
        dtype=np.float32)
    dww = np.concatenate([mp["dw1"].reshape(HID, 1), mp["dw2"].reshape(HID, 9),
                          mp["dw3"].reshape(HID, 25), mp["dw4"].reshape(HID, 49)],
                         axis=1).astype(np.float32)
    dww_r = np.ascontiguousarray(dww.reshape(2, 128, 84).transpose(1, 0, 2).reshape(128, 2 * 84))
    dwb = np.stack([mp["db1"], mp["db2"], mp["db3"], mp["db4"]], axis=1).astype(np.float32)
    dwb_r = np.ascontiguousarray(dwb.reshape(2, 128, 4).transpose(1, 0, 2).reshape(128, 2 * 4))
    w3T_r = np.ascontiguousarray(mp["w3"].T.reshape(8, 128, C).transpose(1, 0, 2).reshape(128, 8 * C))

    a3p_full = np.zeros((B, H + 6, W3PAD), np.float32)
    b3p_full = np.zeros((B, H + 6, W3PAD), np.float32)
    a3p_full[:, 3:3 + H, 3:3 + W] = a3f
    b3p_full[:, 3:3 + H, 3:3 + W] = b3f
    convp_full = np.zeros((B, C, H + 6, W3PAD), np.float32)
    convp_full[:, :, 3:3 + H, 3:3 + W] = convout
    mask_full = np.zeros((H + 6, W3PAD), np.float32)
    mask_full[3:3 + H, 3:3 + W] = 1.0

    in_maps3 = []
    for cj in range(NC_):
        r0 = SL * cj
        in_maps3.append({
            "convp": np.ascontiguousarray(convp_full[:, :, r0:r0 + ROWS3, :]).reshape(B, C, PIX3),
            "a3": np.ascontiguousarray(a3p_full[:, r0:r0 + ROWS3, :]).reshape(B, 1, PIX3),
            "b3": np.ascontiguousarray(b3p_full[:, r0:r0 + ROWS3, :]).reshape(B, 1, PIX3),
            "maskr": np.ascontiguousarray(mask_full[r0:r0 + ROWS3, :]).reshape(1, PIX3),
            "w1T": w1T, "dww": dww_r, "dwb": dwb_r, "w3T": w3T_r,
            "b3c": np.ascontiguousarray(mp["b3"][:, None]),
        })
    r3 = run_bass_kernel_spmd(_build_k3(), in_maps3, core_ids=list(range(NC_))).results

    output = np.empty((B, C, H, W), np.float32)
    for cj in range(NC_):
        output[:, :, SL * cj:SL * cj + SL, :] = r3[cj]["outp"].reshape(B, C, OUTR, W)

    return output, amap_h, amap_w


# revision 11
# speedup vs baseline: 2.0497x; 1.5043x over previous
"""Trainium2 Bass kernel for nn_Block1 (axial cross-attention block).

Sharding: pure data-parallel across 8 NeuronCores.
  - attn_h: each core takes a 16-wide w-range (all b, full h).
  - attn_w: each core takes a 16-tall h-range (all b, full w).
  - conv3x3 + MLP: each core takes a 16-tall h-range of output rows.
Three NEFF launches with host-side resharding between them.

Device kernels keep softmax un-normalized: exp(scores^T) [j,i] tiles are
DMA'd out directly plus per-row reciprocals; the host does amap = exp.T * rcp.
LayerNorm stats (rstd/-mu*rstd per pixel) are computed host-side (a tiny
reduction); the apply (x*alpha + beta) runs on-device (DVE mult + DMA
accumulate-add). LN gamma/beta are folded into consuming conv weights.
"""

import functools
import numpy as np

import concourse.bass as bass
import concourse.tile as tile
from concourse import mybir
from concourse.bass_utils import run_bass_kernel_spmd

# ---- walrus sync-wait workaround ----
import bass_rust
from concourse.vector_clock import ScopedClock

_MAXW = 1
_patch_state = {"done": False, "ctr": 0}


def _mk_nop(engine, wait):
    nop = bass_rust.InstNoOp(name=f"I-waitsplit-{_patch_state['ctr']}", ins=[], outs=[])
    _patch_state["ctr"] += 1
    nop.engine = engine
    nop.sync_info = bass_rust.SyncInfo(on_wait=[wait], on_update=[])
    return nop


def _split_list(insts):
    out = []
    changed = False
    for inst in insts:
        si = getattr(inst, "sync_info", None)
        waits = list(si.on_wait) if (si is not None and si.on_wait) else []
        if len(waits) > _MAXW:
            for w in waits[:-_MAXW]:
                out.append(_mk_nop(inst.engine, w))
            si.on_wait = waits[-_MAXW:]
            changed = True
        out.append(inst)
    if changed:
        insts[:] = out


def _patch_tile():
    if _patch_state["done"]:
        return
    _patch_state["done"] = True

    orig_lower = tile.TileContext._lower_ordered_insts

    def _lower_ordered_insts(self, ordered):
        for insts in ordered.values():
            _split_list(insts)
        return orig_lower(self, ordered)

    tile.TileContext._lower_ordered_insts = _lower_ordered_insts

    def _drain_and_barrier(self, tick_clock, wait_clock):
        drain_inst = self.nc.sync.drain()
        wait_clock.add_sem_waits(
            drain_inst.ins, ScopedClock({None: tick_clock.global_clock})
        )
        si = drain_inst.ins.sync_info
        waits = list(si.on_wait or [])
        if len(waits) > 1:
            si.on_wait = waits[:1]
            for w in waits[1:]:
                d2 = self.nc.sync.drain()
                si2 = d2.ins.sync_info
                if si2 is None:
                    d2.ins.sync_info = bass_rust.SyncInfo(on_wait=[w], on_update=[])
                else:
                    si2.on_wait = [w]

        self.nc.all_engine_barrier()
        assert self.sems is not None
        popped = self.nc._tile_sem_poison_stack.pop()
        assert popped is self._sem_poison
        self.nc.clear_and_free_semaphores(list(self.sems.allocated().values()))
        self.nc.all_engine_barrier()

    tile.TileContext._drain_and_barrier = _drain_and_barrier


_patch_tile()


def _install_neff_cache():
    """Disk-cache walrus NEFF compiles keyed on the BIR json hash."""
    import hashlib
    import os
    import shutil
    from concourse import bass2jax

    if getattr(bass2jax, "_neff_cache_installed", False):
        return
    bass2jax._neff_cache_installed = True
    orig = bass2jax.compile_bir_kernel
    cache_dir = os.path.expanduser("~/.bass_neff_cache")
    os.makedirs(cache_dir, exist_ok=True)

    def cached(bir_json, tmpdir, neff_name="file.neff"):
        key = hashlib.sha256(bir_json).hexdigest()
        p = os.path.join(cache_dir, key + ".neff")
        tgt = os.path.join(tmpdir, neff_name)
        if os.path.exists(p):
            shutil.copy(p, tgt)
            return tgt
        f = orig(bir_json, tmpdir, neff_name=neff_name)
        try:
            shutil.copy(f, p)
        except OSError:
            pass
        return f

    bass2jax.compile_bir_kernel = cached


_install_neff_cache()

# ---- problem constants ----
B, C, H, W = 4, 64, 128, 128
HEADS, DH = 8, 8
HID = 256
NC_ = 8
SL = 16
EPS = 1e-5
SCALE = C ** -0.5
F32 = mybir.dt.float32
BF16 = mybir.dt.bfloat16
AL = mybir.AluOpType
AF = mybir.ActivationFunctionType

PIX1 = H * SL              # 2048
ROWS3 = 22
W3PAD = W + 6              # 134
PIX3 = ROWS3 * W3PAD       # 2948
OUTR = 16
CATR = 24
CATW = 130


def _bc(t, parts, n, offset=0):
    return bass.AP(t, offset, [[0, parts], [1, n]])


def _fap(tile_t, nparts, offset, dims):
    """AP into a pool tile: partitions [0, nparts), free dims given explicitly."""
    a = tile_t[:nparts, :]
    return bass.AP(a.tensor, a.offset + offset, [a.ap[0]] + dims)


def _pslice(tile_t, p0, np_, offset, dims):
    """AP into tile partitions [p0, p0+np_) with explicit free dims."""
    a = tile_t[p0:p0 + np_, :]
    return bass.AP(a.tensor, a.offset + offset, [a.ap[0]] + dims)


# ================= kernel 1: layernorms + both attention branches ==========

@functools.lru_cache(maxsize=1)
def _build_k1():
    nc = bass.Bass()

    ins = {}
    for br in ("h", "w"):
        ins[f"xq_{br}"] = nc.dram_tensor(f"xq_{br}", [B, C, PIX1], F32, kind="ExternalInput")
        ins[f"xkv_{br}"] = nc.dram_tensor(f"xkv_{br}", [B, C, PIX1], F32, kind="ExternalInput")
        for nm in ("aq", "bq", "akv", "bkv"):
            ins[f"{nm}_{br}"] = nc.dram_tensor(f"{nm}_{br}", [B, 1, PIX1], F32, kind="ExternalInput")
        for nm in ("qTs", "kTs"):
            ins[f"{nm}_{br}"] = nc.dram_tensor(f"{nm}_{br}", [C + 1, 2 * 128], BF16, kind="ExternalInput")
        for nm in ("vT", "oT"):
            ins[f"{nm}_{br}"] = nc.dram_tensor(f"{nm}_{br}", [C + 1, C], BF16, kind="ExternalInput")
    ident = nc.dram_tensor("ident", [128, 128], F32, kind="ExternalInput")

    outs = {}
    for br in ("h", "w"):
        outs[f"eamap_{br}"] = nc.dram_tensor(f"eamap_{br}", [B, SL, HEADS, 128, 128], BF16, kind="ExternalOutput")
        outs[f"rcp_{br}"] = nc.dram_tensor(f"rcp_{br}", [B, SL, 128, HEADS], F32, kind="ExternalOutput")
        outs[f"outres_{br}"] = nc.dram_tensor(f"outres_{br}", [B, C, PIX1], F32, kind="ExternalOutput")

    with tile.TileContext(nc) as tc:
        import contextlib
        ctx = contextlib.ExitStack()
        with ctx:
            const = ctx.enter_context(tc.tile_pool(name="const", bufs=1))
            lnp = ctx.enter_context(tc.tile_pool(name="lnp", bufs=2))
            sbp = ctx.enter_context(tc.tile_pool(name="sbp", bufs=3))
            expp = ctx.enter_context(tc.tile_pool(name="expp", bufs=4))
            ps_qk = ctx.enter_context(tc.tile_pool(name="ps_qk", bufs=2, space="PSUM"))
            ps_vo = ctx.enter_context(tc.tile_pool(name="ps_vo", bufs=1, space="PSUM"))
            ps_sT = ctx.enter_context(tc.tile_pool(name="ps_sT", bufs=2, space="PSUM"))
            ps_sm = ctx.enter_context(tc.tile_pool(name="ps_sm", bufs=1, space="PSUM"))
            ps_oa = ctx.enter_context(tc.tile_pool(name="ps_oa", bufs=1, space="PSUM"))

            ident_sb = const.tile([128, 128], F32)
            nc.sync.dma_start(out=ident_sb, in_=ident[:, :])

            wsb = {}
            for br in ("h", "w"):
                for nm in ("qTs", "kTs"):
                    t = const.tile([C + 1, 2 * 128], BF16, tag=f"w_{nm}_{br}")
                    nc.sync.dma_start(out=t, in_=ins[f"{nm}_{br}"][:, :])
                    wsb[f"{nm}_{br}"] = t
                for nm in ("vT", "oT"):
                    t = const.tile([C + 1, C], BF16, tag=f"w_{nm}_{br}")
                    nc.sync.dma_start(out=t, in_=ins[f"{nm}_{br}"][:, :])
                    wsb[f"{nm}_{br}"] = t

            for b in range(B):
                for br in ("h", "w"):
                    if br == "h":
                        pstep, sstep = SL, 1      # col = 16*pix + s
                    else:
                        pstep, sstep = 1, 128     # col = 128*s + pix

                    # ---- LN applies ----
                    xq_raw = lnp.tile([C, PIX1], F32, tag="xq_raw")
                    nc.sync.dma_start(out=xq_raw, in_=ins[f"xq_{br}"][b, :, :])
                    a_bc = lnp.tile([C, PIX1], F32, tag="a_bc")
                    nc.sync.dma_start(out=a_bc, in_=_bc(ins[f"aq_{br}"], C, PIX1, b * PIX1))
                    xnq = lnp.tile([C + 1, PIX1], BF16, tag="xnq")
                    nc.vector.tensor_mul(out=xnq[:C, :], in0=xq_raw, in1=a_bc)
                    nc.gpsimd.dma_start(out=xnq[:C, :], in_=_bc(ins[f"bq_{br}"], C, PIX1, b * PIX1), accum_op=AL.add)
                    nc.vector.memset(xnq[C:C + 1, :], 1.0)

                    xkv_raw = lnp.tile([C, PIX1], F32, tag="xkv_raw")
                    nc.sync.dma_start(out=xkv_raw, in_=ins[f"xkv_{br}"][b, :, :])
                    akv_bc = lnp.tile([C, PIX1], F32, tag="akv_bc")
                    nc.sync.dma_start(out=akv_bc, in_=_bc(ins[f"akv_{br}"], C, PIX1, b * PIX1))
                    xnkv = lnp.tile([C + 1, PIX1], BF16, tag="xnkv")
                    nc.vector.tensor_mul(out=xnkv[:C, :], in0=xkv_raw, in1=akv_bc)
                    nc.gpsimd.dma_start(out=xnkv[:C, :], in_=_bc(ins[f"bkv_{br}"], C, PIX1, b * PIX1), accum_op=AL.add)
                    nc.vector.memset(xnkv[C:C + 1, :], 1.0)

                    qT_w, kT_w, vT_w, oT_w = (wsb[f"{nm}_{br}"] for nm in ("qTs", "kTs", "vT", "oT"))

                    for g in range(SL // 4):
                        s0 = 4 * g
                        rhs_q = _fap(xnq, C + 1, s0 * sstep, [[sstep, 4], [pstep, 128]])
                        rhs_kv = _fap(xnkv, C + 1, s0 * sstep, [[sstep, 4], [pstep, 128]])

                        qk_sb = {}
                        for nm, rhs, wT in (("q", rhs_q, qT_w), ("k", rhs_kv, kT_w)):
                            for half in range(2):
                                ps = ps_qk.tile([128, 512], F32, tag="qkspread")
                                nc.tensor.matmul(ps, wT[:, 128 * half:128 * half + 128],
                                                 rhs, start=True, stop=True)
                                sb = sbp.tile([128, 512], BF16, tag=f"{nm}sb{half}")
                                if nm == "q":
                                    nc.scalar.copy(out=sb, in_=ps)
                                else:
                                    nc.vector.tensor_copy(out=sb, in_=ps)
                                qk_sb[(nm, half)] = sb

                        v_ps = ps_vo.tile([C, 512], F32, tag="vps")
                        nc.tensor.matmul(v_ps, vT_w, rhs_kv, start=True, stop=True)
                        v_sb = sbp.tile([C, 512], F32, tag="vsb")
                        nc.vector.tensor_copy(out=v_sb, in_=v_ps)

                        o_ext = sbp.tile([C + 1, 512], BF16, tag="oext")
                        nc.vector.memset(o_ext[C:C + 1, :], 1.0)

                        for ds in range(4):
                            s = s0 + ds
                            vT_ps = ps_sm.tile([128, C], F32, tag="small")
                            nc.tensor.transpose(vT_ps, _fap(v_sb, C, 128 * ds, [[1, 128]]), ident_sb[:C, :C])
                            vT_ext = sbp.tile([128, 9 * HEADS], BF16, tag="vText")
                            nc.vector.memset(_fap(vT_ext, 128, 8, [[9, 8]]), 1.0)
                            nc.vector.tensor_copy(out=_fap(vT_ext, 128, 0, [[9, 8], [1, 8]]), in_=vT_ps)

                            o_all = ps_oa.tile([128, 9 * HEADS], F32, tag="oall")
                            for n_head in range(HEADS):
                                half, m = divmod(n_head, 4)
                                lhs = _pslice(qk_sb[("k", half)], 32 * m, 8, 128 * ds, [[1, 128]])
                                rhs = _pslice(qk_sb[("q", half)], 32 * m, 8, 128 * ds, [[1, 128]])
                                sT_ps = ps_sT.tile([128, 128], F32, tag="sT")
                                nc.tensor.matmul(sT_ps, lhs, rhs, start=True, stop=True,
                                                 tile_position=(32 * m, 0))
                                expT = expp.tile([128, 128], BF16, tag="expT")
                                nc.scalar.activation(out=expT, in_=sT_ps, func=AF.Exp, scale=SCALE)
                                nc.sync.dma_start(out=outs[f"eamap_{br}"][b, s, n_head, :, :], in_=expT)
                                nc.tensor.matmul(o_all[:, 9 * n_head:9 * n_head + 9], expT,
                                                 vT_ext[:, 9 * n_head:9 * n_head + 9],
                                                 start=True, stop=True)

                            rcp_sb = sbp.tile([128, HEADS], F32, tag="rcp")
                            nc.vector.reciprocal(out=rcp_sb, in_=_fap(o_all, 128, 8, [[9, 8]]))
                            nc.sync.dma_start(out=outs[f"rcp_{br}"][b, s, :, :], in_=rcp_sb)

                            oT_sb = sbp.tile([128, C], F32, tag="oTsb")
                            nc.vector.tensor_mul(
                                out=_fap(oT_sb, 128, 0, [[8, 8], [1, 8]]),
                                in0=_fap(o_all, 128, 0, [[9, 8], [1, 8]]),
                                in1=_fap(rcp_sb, 128, 0, [[1, 8], [0, 8]]),
                            )

                            o_ps2 = ps_sm.tile([C, 128], F32, tag="small")
                            nc.tensor.transpose(o_ps2, oT_sb, ident_sb)
                            nc.scalar.copy(out=o_ext[:C, 128 * ds:128 * ds + 128], in_=o_ps2)

                        oh_ps = ps_vo.tile([C, 512], F32, tag="ohps")
                        nc.tensor.matmul(oh_ps, oT_w, o_ext, start=True, stop=True)
                        oh_sb = sbp.tile([C, 512], F32, tag="ohsb")
                        nc.vector.tensor_add(out=oh_sb, in0=oh_ps,
                                             in1=_fap(xkv_raw, C, s0 * sstep, [[sstep, 4], [pstep, 128]]))
                        dst = bass.AP(outs[f"outres_{br}"], b * C * PIX1 + 512 * g,
                                      [[PIX1, C], [1, 512]])
                        nc.sync.dma_start(out=dst, in_=oh_sb)
    return nc


# ================= kernel 2: 3x3 conv over concat(out_h, out_w) ============

@functools.lru_cache(maxsize=1)
def _build_k2():
    nc = bass.Bass()
    catp = nc.dram_tensor("catp", [B, 2 * C, CATR * CATW], F32, kind="ExternalInput")
    cwT = nc.dram_tensor("cwT", [2 * C, 9 * C], F32, kind="ExternalInput")
    cb = nc.dram_tensor("cb", [C, 1], F32, kind="ExternalInput")
    convout = nc.dram_tensor("convout", [B, C, ROWS3 * W], F32, kind="ExternalOutput")

    with tile.TileContext(nc) as tc:
        import contextlib
        ctx = contextlib.ExitStack()
        with ctx:
            const = ctx.enter_context(tc.tile_pool(name="const", bufs=1))
            sbp = ctx.enter_context(tc.tile_pool(name="sbp", bufs=2))
            psp = ctx.enter_context(tc.tile_pool(name="psp", bufs=4, space="PSUM"))

            cw_sb = const.tile([2 * C, 9 * C], F32)
            nc.sync.dma_start(out=cw_sb, in_=cwT[:, :])
            cb_sb = const.tile([C, 1], F32)
            nc.sync.dma_start(out=cb_sb, in_=cb[:, :])

            for b in range(B):
                x_sb = sbp.tile([2 * C, CATR * CATW], F32, tag="x")
                nc.sync.dma_start(out=x_sb, in_=catp[b, :, :])
                out_sb = sbp.tile([C, ROWS3 * W], F32, tag="o")
                for r0 in range(0, ROWS3, 4):
                    nr = min(4, ROWS3 - r0)
                    ps = psp.tile([C, 512], F32, tag="ps")
                    ti = 0
                    for dy in (-1, 0, 1):
                        for dx in (-1, 0, 1):
                            rhs = _fap(x_sb, 2 * C, (r0 + 1 + dy) * CATW + 1 + dx,
                                       [[CATW, nr], [1, W]])
                            nc.tensor.matmul(ps[:, :nr * W],
                                             cw_sb[:, (ti * C):(ti * C + C)],
                                             rhs, start=(ti == 0), stop=(ti == 8))
                            ti += 1
                    nc.vector.tensor_scalar_add(out=out_sb[:, r0 * W:(r0 + nr) * W],
                                                in0=ps[:, :nr * W], scalar1=cb_sb)
                nc.sync.dma_start(out=convout[b, :, :], in_=out_sb)
    return nc


# ================= kernel 3: LN3 + MLP =====================================

@functools.lru_cache(maxsize=1)
def _build_k3():
    nc = bass.Bass()
    convp = nc.dram_tensor("convp", [B, C, PIX3], F32, kind="ExternalInput")
    a3 = nc.dram_tensor("a3", [B, 1, PIX3], F32, kind="ExternalInput")
    b3 = nc.dram_tensor("b3", [B, 1, PIX3], F32, kind="ExternalInput")
    maskr = nc.dram_tensor("maskr", [1, PIX3], F32, kind="ExternalInput")
    w1T = nc.dram_tensor("w1T", [C + 1, HID], BF16, kind="ExternalInput")
    dww = nc.dram_tensor("dww", [128, 2 * 84], F32, kind="ExternalInput")
    dwb = nc.dram_tensor("dwb", [128, 2 * 4], F32, kind="ExternalInput")
    # dw4 diagonal stationaries: [128, (hf,tap)*128] bf16, col-block hf*49+tap
    dwdiag = nc.dram_tensor("dwdiag", [128, 2 * 49 * 128], BF16, kind="ExternalInput")
    w3T = nc.dram_tensor("w3T", [128, 8 * C], BF16, kind="ExternalInput")
    b3c = nc.dram_tensor("b3c", [C, 1], F32, kind="ExternalInput")
    outp = nc.dram_tensor("outp", [B, C, OUTR * W], F32, kind="ExternalOutput")

    NPIX = OUTR * W
    taps = {}
    for k in (3, 5, 7):
        hw = k // 2
        taps[k] = [(dy, dx) for dy in range(-hw, hw + 1) for dx in range(-hw, hw + 1)]
    twoff = {1: 0, 3: 1, 5: 10, 7: 35}   # tap column offsets within 84

    with tile.TileContext(nc) as tc:
        import contextlib
        ctx = contextlib.ExitStack()
        with ctx:
            const = ctx.enter_context(tc.tile_pool(name="const", bufs=1))
            lnp = ctx.enter_context(tc.tile_pool(name="lnp", bufs=1))
            zp = ctx.enter_context(tc.tile_pool(name="zp", bufs=1))
            gp = ctx.enter_context(tc.tile_pool(name="gp", bufs=1))
            psp = ctx.enter_context(tc.tile_pool(name="psp", bufs=2, space="PSUM"))
            ps4 = ctx.enter_context(tc.tile_pool(name="ps4", bufs=2, space="PSUM"))
            pso = ctx.enter_context(tc.tile_pool(name="pso", bufs=2, space="PSUM"))

            w1_sb = const.tile([C + 1, HID], BF16)
            nc.sync.dma_start(out=w1_sb, in_=w1T[:, :])
            w3_sb = const.tile([128, 8 * C], BF16)
            nc.sync.dma_start(out=w3_sb, in_=w3T[:, :])
            dww_sb = const.tile([128, 2 * 84], F32)
            nc.sync.dma_start(out=dww_sb, in_=dww[:, :])
            dwb_sb = const.tile([128, 2 * 4], F32)
            nc.sync.dma_start(out=dwb_sb, in_=dwb[:, :])
            dwdiag_sb = const.tile([128, 2 * 49 * 128], BF16)
            nc.sync.dma_start(out=dwdiag_sb, in_=dwdiag[:, :])
            b3c_sb = const.tile([C, 1], F32)
            nc.sync.dma_start(out=b3c_sb, in_=b3c[:, :])

            for b in range(B):
                x_sb = lnp.tile([C, PIX3], F32, tag="x")
                nc.sync.dma_start(out=x_sb, in_=convp[b, :, :])
                a_bc = lnp.tile([C, PIX3], F32, tag="abc")
                nc.sync.dma_start(out=a_bc, in_=_bc(a3, C, PIX3, b * PIX3))
                xn = lnp.tile([C + 1, PIX3], BF16, tag="xn")
                nc.vector.tensor_mul(out=xn[:C, :], in0=x_sb, in1=a_bc)
                nc.gpsimd.dma_start(out=xn[:C, :], in_=_bc(b3, C, PIX3, b * PIX3), accum_op=AL.add)
                nc.gpsimd.dma_start(out=xn[C:C + 1, :], in_=maskr[:, :])

                z = []
                for hf in range(2):
                    zt = zp.tile([128, PIX3], BF16, tag=f"z{hf}")
                    for c0 in range(0, PIX3, 512):
                        nn_ = min(512, PIX3 - c0)
                        ps = psp.tile([128, 512], F32, tag="zps")
                        nc.tensor.matmul(ps[:, :nn_], w1_sb[:, 128 * hf:128 * hf + 128],
                                         xn[:, c0:c0 + nn_], start=True, stop=True)
                        nc.scalar.activation(out=zt[:, c0:c0 + nn_], in_=ps[:, :nn_],
                                             func=AF.Gelu)
                    z.append(zt)

                gtiles = []
                for ki, k in enumerate((1, 3, 5, 7)):
                    for hf in range(2):
                        gt = gp.tile([128, NPIX], BF16, tag=f"g{ki}{hf}")
                        zt = z[hf]
                        wcol0 = 84 * hf + twoff[k]

                        def zin(dy, dx, r0=0, nr=OUTR):
                            return _fap(zt, 128, (3 + r0 + dy) * W3PAD + 3 + dx,
                                        [[W3PAD, nr], [1, W]])

                        if k == 1:
                            nc.scalar.activation(out=gt, in_=zin(0, 0), func=AF.Gelu,
                                                 scale=dww_sb[:, wcol0:wcol0 + 1],
                                                 bias=dwb_sb[:, 4 * hf + ki:4 * hf + ki + 1])
                        elif k == 7:
                            # dw4 on PE: diag-stationary matmuls, fp32 psum acc
                            for r0 in range(0, OUTR, 4):
                                ps = ps4.tile([128, 512], F32, tag="dw4ps")
                                for ti, (dy, dx) in enumerate(taps[k]):
                                    dcol = (hf * 49 + ti) * 128
                                    nc.tensor.matmul(ps, dwdiag_sb[:, dcol:dcol + 128],
                                                     zin(dy, dx, r0, 4),
                                                     start=(ti == 0), stop=(ti == 48))
                                nc.scalar.activation(out=gt[:, r0 * W:(r0 + 4) * W], in_=ps,
                                                     func=AF.Gelu,
                                                     bias=dwb_sb[:, 4 * hf + ki:4 * hf + ki + 1],
                                                     scale=1.0)
                        else:
                            acc = gp.tile([128, NPIX], BF16, tag="acc")
                            eng = nc.vector
                            for ti, (dy, dx) in enumerate(taps[k]):
                                wcol = dww_sb[:, wcol0 + ti:wcol0 + ti + 1]
                                if ti == 0:
                                    eng.tensor_scalar_mul(out=acc, in0=zin(dy, dx), scalar1=wcol)
                                else:
                                    eng.scalar_tensor_tensor(out=acc, in0=zin(dy, dx), scalar=wcol,
                                                             in1=acc, op0=AL.mult, op1=AL.add)
                            nc.scalar.activation(out=gt, in_=acc, func=AF.Gelu,
                                                 bias=dwb_sb[:, 4 * hf + ki:4 * hf + ki + 1],
                                                 scale=1.0)
                        gtiles.append(gt)
                out_sb = gp.tile([C, NPIX], F32, tag="out")
                for c0 in range(0, NPIX, 512):
                    ps = pso.tile([C, 512], F32, tag="ops")
                    for kc in range(8):
                        nc.tensor.matmul(ps, w3_sb[:, C * kc:C * kc + C],
                                         gtiles[kc][:, c0:c0 + 512],
                                         start=(kc == 0), stop=(kc == 7))
                    res = _fap(x_sb, C, 3 * W3PAD + 3 + (c0 // W) * W3PAD, [[W3PAD, 4], [1, W]])
                    nc.vector.scalar_tensor_tensor(out=out_sb[:, c0:c0 + 512], in0=ps,
                                                   scalar=b3c_sb, in1=res,
                                                   op0=AL.add, op1=AL.add)
                nc.sync.dma_start(out=outp[b, :, :], in_=out_sb)
    return nc


# ======================= host orchestration ================================

def _ln_ab(x):
    mu = x.mean(axis=1)
    var = x.var(axis=1)
    rstd = 1.0 / np.sqrt(var + EPS)
    return rstd.astype(np.float32), (-mu * rstd).astype(np.float32)


BF16NP = mybir.dt.np(mybir.dt.bfloat16)


def _wpack_spread(wn, bn, g, bln):
    """[C+1, 2*128] bf16: half h, col 32m+d = (head 4h+m, dim d); zeros else."""
    w_eff = wn * g[None, :]
    b_eff = wn @ bln + bn
    full = np.concatenate([w_eff.T, b_eff[None, :]], axis=0)  # [65, 64]
    out = np.zeros((C + 1, 2 * 128), np.float32)
    for n in range(HEADS):
        half, m = divmod(n, 4)
        out[:, 128 * half + 32 * m:128 * half + 32 * m + DH] = full[:, DH * n:DH * n + DH]
    return np.ascontiguousarray(out.astype(BF16NP))


def _wpack(wn, bn, g, bln):
    w_eff = wn * g[None, :]
    b_eff = wn @ bln + bn
    return np.ascontiguousarray(
        np.concatenate([w_eff.T, b_eff[None, :]], axis=0), dtype=np.float32)


def kernel(pan, lms, params):
    pan = np.asarray(pan, dtype=np.float32)
    lms = np.asarray(lms, dtype=np.float32)
    p = {k: (np.asarray(v, dtype=np.float32) if not isinstance(v, dict) else
             {k2: np.asarray(v2, dtype=np.float32) for k2, v2 in v.items()})
         for k, v in params.items()}

    g1, b1 = p["ln1_g"], p["ln1_b"]
    g2, b2 = p["ln2_g"], p["ln2_b"]
    a_pan, be_pan = _ln_ab(pan)
    a_lms, be_lms = _ln_ab(lms)
    ident = np.eye(128, dtype=np.float32)

    in_maps = []
    for cj in range(NC_):
        ws = SL * cj
        m = {"ident": ident}
        for br, ap_ in (("h", p["attn_h"]), ("w", p["attn_w"])):
            m[f"qTs_{br}"] = _wpack_spread(ap_["wq"], ap_["bq"], g1, b1)
            m[f"kTs_{br}"] = _wpack_spread(ap_["wk"], ap_["bk"], g2, b2)
            m[f"vT_{br}"] = _wpack(ap_["wv"], ap_["bv"], g2, b2).astype(BF16NP)
            m[f"oT_{br}"] = np.ascontiguousarray(
                np.concatenate([ap_["wo"].T, ap_["bo"][None, :]], axis=0)).astype(BF16NP)
        m["xq_h"] = np.ascontiguousarray(pan[:, :, :, ws:ws + SL]).reshape(B, C, PIX1)
        m["xkv_h"] = np.ascontiguousarray(lms[:, :, :, ws:ws + SL]).reshape(B, C, PIX1)
        m["aq_h"] = np.ascontiguousarray(a_pan[:, :, ws:ws + SL]).reshape(B, 1, PIX1)
        m["bq_h"] = np.ascontiguousarray(be_pan[:, :, ws:ws + SL]).reshape(B, 1, PIX1)
        m["akv_h"] = np.ascontiguousarray(a_lms[:, :, ws:ws + SL]).reshape(B, 1, PIX1)
        m["bkv_h"] = np.ascontiguousarray(be_lms[:, :, ws:ws + SL]).reshape(B, 1, PIX1)
        m["xq_w"] = np.ascontiguousarray(pan[:, :, ws:ws + SL, :]).reshape(B, C, PIX1)
        m["xkv_w"] = np.ascontiguousarray(lms[:, :, ws:ws + SL, :]).reshape(B, C, PIX1)
        m["aq_w"] = np.ascontiguousarray(a_pan[:, ws:ws + SL, :]).reshape(B, 1, PIX1)
        m["bq_w"] = np.ascontiguousarray(be_pan[:, ws:ws + SL, :]).reshape(B, 1, PIX1)
        m["akv_w"] = np.ascontiguousarray(a_lms[:, ws:ws + SL, :]).reshape(B, 1, PIX1)
        m["bkv_w"] = np.ascontiguousarray(be_lms[:, ws:ws + SL, :]).reshape(B, 1, PIX1)
        in_maps.append(m)

    r1 = run_bass_kernel_spmd(_build_k1(), in_maps, core_ids=list(range(NC_))).results

    out_h = np.empty((B, C, H, W), np.float32)
    out_w = np.empty((B, C, H, W), np.float32)
    amap_h = np.empty((B * W, HEADS, H, H), np.float32)
    amap_w = np.empty((B * H, HEADS, W, W), np.float32)
    ah5 = amap_h.reshape(B, NC_, SL, HEADS, H, H)
    aw5 = amap_w.reshape(B, NC_, SL, HEADS, W, W)
    for cj in range(NC_):
        ws = SL * cj
        res = r1[cj]
        out_h[:, :, :, ws:ws + SL] = res["outres_h"].reshape(B, C, SL, H).transpose(0, 1, 3, 2)
        out_w[:, :, ws:ws + SL, :] = res["outres_w"].reshape(B, C, SL, W)
        for br, dst in (("h", ah5), ("w", aw5)):
            e = res[f"eamap_{br}"].astype(np.float32)
            rc = res[f"rcp_{br}"]
            dst[:, cj] = e.transpose(0, 1, 2, 4, 3) * rc.transpose(0, 1, 3, 2)[:, :, :, :, None]

    cat = np.concatenate([out_h, out_w], axis=1)
    catp_full = np.zeros((B, 2 * C, H + 8, CATW), np.float32)
    catp_full[:, :, 4:4 + H, 1:1 + W] = cat
    cw = p["conv_w"]
    cwT = np.ascontiguousarray(cw.transpose(1, 2, 3, 0).reshape(2 * C, 9 * C))
    in_maps2 = []
    for cj in range(NC_):
        r0 = SL * cj
        in_maps2.append({
            "catp": np.ascontiguousarray(catp_full[:, :, r0:r0 + CATR, :]).reshape(B, 2 * C, CATR * CATW),
            "cwT": cwT,
            "cb": np.ascontiguousarray(p["conv_b"][:, None]),
        })
    r2 = run_bass_kernel_spmd(_build_k2(), in_maps2, core_ids=list(range(NC_))).results

    convout = np.empty((B, C, H, W), np.float32)
    for cj in range(NC_):
        co = r2[cj]["convout"].reshape(B, C, ROWS3, W)
        convout[:, :, SL * cj:SL * cj + SL, :] = co[:, :, 3:3 + SL, :]

    g3, bl3 = p["ln3_g"], p["ln3_b"]
    a3f, b3f = _ln_ab(convout)
    mp = p["mlp"]
    w1T = np.ascontiguousarray(np.concatenate(
        [(mp["w1"] * g3[None, :]).T, (mp["w1"] @ bl3 + mp["b1"])[None, :]], axis=0),
        dtype=np.float32).astype(BF16NP)
    dww = np.concatenate([mp["dw1"].reshape(HID, 1), mp["dw2"].reshape(HID, 9),
                          mp["dw3"].reshape(HID, 25), mp["dw4"].reshape(HID, 49)],
                         axis=1).astype(np.float32)
    dww_r = np.ascontiguousarray(dww.reshape(2, 128, 84).transpose(1, 0, 2).reshape(128, 2 * 84))
    dwb = np.stack([mp["db1"], mp["db2"], mp["db3"], mp["db4"]], axis=1).astype(np.float32)
    dwb_r = np.ascontiguousarray(dwb.reshape(2, 128, 4).transpose(1, 0, 2).reshape(128, 2 * 4))
    w3T_r = np.ascontiguousarray(mp["w3"].T.reshape(8, 128, C).transpose(1, 0, 2).reshape(128, 8 * C)).astype(BF16NP)
    # dw4 diag stationaries [128, (hf*49+tap)*128] bf16
    dw4w = mp["dw4"].reshape(HID, 49)  # [256, 49]
    dwdiag = np.zeros((128, 2 * 49 * 128), np.float32)
    idx = np.arange(128)
    for hf in range(2):
        for t in range(49):
            dwdiag[idx, (hf * 49 + t) * 128 + idx] = dw4w[128 * hf:128 * hf + 128, t]
    dwdiag = np.ascontiguousarray(dwdiag.astype(BF16NP))

    a3p_full = np.zeros((B, H + 6, W3PAD), np.float32)
    b3p_full = np.zeros((B, H + 6, W3PAD), np.float32)
    a3p_full[:, 3:3 + H, 3:3 + W] = a3f
    b3p_full[:, 3:3 + H, 3:3 + W] = b3f
    convp_full = np.zeros((B, C, H + 6, W3PAD), np.float32)
    convp_full[:, :, 3:3 + H, 3:3 + W] = convout
    mask_full = np.zeros((H + 6, W3PAD), np.float32)
    mask_full[3:3 + H, 3:3 + W] = 1.0

    in_maps3 = []
    for cj in range(NC_):
        r0 = SL * cj
        in_maps3.append({
            "convp": np.ascontiguousarray(convp_full[:, :, r0:r0 + ROWS3, :]).reshape(B, C, PIX3),
            "a3": np.ascontiguousarray(a3p_full[:, r0:r0 + ROWS3, :]).reshape(B, 1, PIX3),
            "b3": np.ascontiguousarray(b3p_full[:, r0:r0 + ROWS3, :]).reshape(B, 1, PIX3),
            "maskr": np.ascontiguousarray(mask_full[r0:r0 + ROWS3, :]).reshape(1, PIX3),
            "w1T": w1T, "dww": dww_r, "dwb": dwb_r, "w3T": w3T_r, "dwdiag": dwdiag,
            "b3c": np.ascontiguousarray(mp["b3"][:, None]),
        })
    r3 = run_bass_kernel_spmd(_build_k3(), in_maps3, core_ids=list(range(NC_))).results

    output = np.empty((B, C, H, W), np.float32)
    for cj in range(NC_):
        output[:, :, SL * cj:SL * cj + SL, :] = r3[cj]["outp"].reshape(B, C, OUTR, W)

    return output, amap_h, amap_w
